# revision 15
# baseline (speedup 1.0000x reference)
import sys

import ml_dtypes
import numpy as np

_TRN_REPO = "/opt/trn_rl_repo"
if _TRN_REPO not in sys.path:
    sys.path.insert(0, _TRN_REPO)

import concourse.tile as tile
from concourse import bacc, mybir
from concourse.bass_utils import run_bass_kernel_spmd

F32 = mybir.dt.float32
F32R = mybir.dt.float32r
BF16 = mybir.dt.bfloat16
AF = mybir.ActivationFunctionType

B, S, D = 2, 2048, 768
H_TOT, W = 12, 64
N_CORES = 8
HL = 3
DH = HL * W
KC = D // 128
ST = 512
NS = S // ST
PT = 1024
NT = S // 128
BF = ml_dtypes.bfloat16


def _round_f32r(a):
    u = np.ascontiguousarray(a, np.float32).view(np.uint32).copy()
    u += np.uint32(0x7FF) + ((u >> np.uint32(12)) & np.uint32(1))
    u &= np.uint32(0xFFFFF000)
    return u.view(np.float32)


def _emit(tc, aps, has_bias, has_mask):
    nc = tc.nc
    xt_d, wq_d, wk_d, wv_d, on_d, o32_d, mb_d, out_d = aps

    CH = 1 if has_mask else 3
    SCW = CH * ST

    from contextlib import ExitStack

    with ExitStack() as ctx:
        const = ctx.enter_context(tc.tile_pool(name="const", bufs=1))

        ones = None
        if has_bias:
            ones = const.tile([1, PT], BF16, name="ones", tag="ones")
        ones_r = const.tile([1, W], F32R, name="ones_r", tag="ones_r")
        mb = None
        if has_mask:
            mb = const.tile([128, NT], F32, name="mb", tag="mb")

        xt = []
        for c in range(KC):
            t = const.tile([128, S], BF16, name=f"xt{c}", tag=f"xt{c}")
            xt.append(t)

        def w_tiles(name):
            chunks = []
            for c in range(KC):
                t = const.tile([128, DH], BF16, name=f"{name}{c}",
                               tag=f"{name}{c}")
                chunks.append(t)
            brow = const.tile([1, DH], BF16, name=f"{name}b", tag=f"{name}b")
            return chunks, brow

        wq, wqb = w_tiles("wq")
        wk, wkb = w_tiles("wk")
        wv, wvb = w_tiles("wv")

        dmae = [nc.sync, nc.gpsimd]
        for c in range(KC):
            dmae[c % 2].dma_start(
                out=xt[c][:, 0:ST], in_=xt_d[c * 128:(c + 1) * 128, 0:ST])
        for c in range(KC):
            dmae[c % 2].dma_start(
                out=wk[c][:], in_=wk_d[c * 128:(c + 1) * 128, :])
            dmae[(c + 1) % 2].dma_start(
                out=wq[c][:], in_=wq_d[c * 128:(c + 1) * 128, :])
        for si in range(1, NS):
            ssl = slice(si * ST, (si + 1) * ST)
            for c in range(KC):
                dmae[(si + c) % 2].dma_start(
                    out=xt[c][:, ssl], in_=xt_d[c * 128:(c + 1) * 128, ssl])
        for c in range(KC):
            dmae[c % 2].dma_start(
                out=wv[c][:], in_=wv_d[c * 128:(c + 1) * 128, :])
        if has_bias:
            for brow, w_d in ((wqb, wq_d), (wkb, wk_d), (wvb, wv_d)):
                nc.sync.dma_start(out=brow[:], in_=w_d[D:D + 1, :])
            nc.sync.dma_start(out=ones[:], in_=on_d[0:1, 0:PT])
        nc.sync.dma_start(out=ones_r[:], in_=o32_d[0:1, :])
        if has_mask:
            nc.sync.dma_start(out=mb[:], in_=mb_d[:, :])

        kt = []
        qt = []
        for h in range(HL):
            kt.append(const.tile([128, S], BF16, name=f"kt{h}", tag=f"kt{h}"))
            qt.append(const.tile([128, S], BF16, name=f"qt{h}", tag=f"qt{h}"))
        vaug = []
        for t in range(NT):
            va = const.tile([128, HL, W + 1], BF16, name=f"vaug{t}",
                            tag=f"vaug{t}")
            nc.gpsimd.memset(va[:, :, W:W + 1], 1.0)
            vaug.append(va)

        strm = ctx.enter_context(
            tc.tile_pool(name="strm", bufs=1, space="PSUM"))
        epi = ctx.enter_context(tc.tile_pool(name="epi", bufs=2))
        exp_pool = ctx.enter_context(tc.tile_pool(name="exp", bufs=6))

        def dup01(chunk, dk, ssl):
            nc.vector.tensor_copy(dk[0][0:64, ssl], chunk[0:64, :])
            nc.vector.tensor_copy(dk[0][64:128, ssl], chunk[0:64, :])
            nc.vector.tensor_copy(dk[1][0:64, ssl], chunk[64:128, :])
            nc.vector.tensor_copy(dk[1][64:128, ssl], chunk[64:128, :])

        def proj_main(si, dst2, brow, wch):
            ssl = slice(si * ST, (si + 1) * ST)
            wrk = strm.tile([128, ST], F32, name="wrk", tag="work", bufs=2)
            for c in range(KC):
                nc.tensor.matmul(
                    wrk[:], wch[c][:, 0:128], xt[c][:, ssl],
                    start=(c == 0), stop=False, skip_group_check=True,
                )
            nc.tensor.matmul(
                wrk[:], brow[:, 0:128], ones[:, 0:ST],
                start=False, stop=True, skip_group_check=True,
            )
            dup01(wrk[:], dst2, ssl)

        def proj_h2_bias(si, wch, brow, dst):
            ssl = slice(si * ST, (si + 1) * ST)
            wrk = strm.tile([128, ST], F32, name="wrkb", tag="work", bufs=2)
            for c in range(KC):
                nc.tensor.matmul(
                    wrk[0:64, :], wch[c][:, 128:DH], xt[c][:, ssl],
                    start=(c == 0), stop=False, skip_group_check=True,
                )
            nc.tensor.matmul(
                wrk[0:64, :], brow[:, 128:DH], ones[:, 0:ST],
                start=False, stop=True, skip_group_check=True,
            )
            nc.vector.tensor_copy(dst[0:64, ssl], wrk[0:64, :])
            nc.vector.tensor_copy(dst[64:128, ssl], wrk[0:64, :])

        if has_bias:
            for si in range(NS):
                proj_main(si, (kt[0], kt[1]), wkb, wk)
                proj_h2_bias(si, wk, wkb, kt[2])
                proj_main(si, (qt[0], qt[1]), wqb, wq)
                proj_h2_bias(si, wq, wqb, qt[2])
            for t in range(NT):
                tsl = slice(t * 128, (t + 1) * 128)
                wrk = strm.tile([128, ST], F32, name="wrkv", tag="work",
                                bufs=2)
                for c in range(KC):
                    nc.tensor.matmul(
                        wrk[:, 0:DH], xt[c][:, tsl], wv[c][:],
                        start=(c == 0), stop=False, skip_group_check=True,
                    )
                nc.tensor.matmul(
                    wrk[:, 0:DH], ones[:, 0:128], wvb[:],
                    start=False, stop=True, skip_group_check=True,
                )
                nc.vector.tensor_copy(
                    vaug[t][:, :, 0:W],
                    wrk[:, 0:DH].rearrange("p (h w) -> p h w", h=HL),
                )

        st_ = {"sc": None, "used": 0, "base": 0, "chunks": [], "tick": 0}
        stash = []
        pend1 = []
        pend2 = []

        def new_sc():
            st_["sc"] = strm.tile([128, SCW], F32, name="sc", tag="sc",
                                  bufs=2)
            st_["used"] = 0
            st_["base"] = 0
            st_["chunks"] = []

        def close_group():
            sc = st_["sc"]
            if sc is None:
                return
            n = len(st_["chunks"])
            if n:
                lo = st_["base"] * ST
                hi = lo + n * ST
                ex = exp_pool.tile([128, SCW], BF16, name="ex", tag="ex")
                tlast = st_["chunks"][-1][2]
                nc.scalar.activation(
                    ex[:, 0:n * ST], sc[:, lo:hi], AF.Exp,
                    bias=(mb[:, tlast:tlast + 1] if has_mask else 0.0),
                    scale=0.125,
                )
                stash.append(
                    (ex, [(cinfo, h, t, i * ST)
                          for i, (cinfo, h, t) in enumerate(st_["chunks"])]))
            st_["sc"] = None

        def emit_stash(drain=False):
            while len(stash) > (0 if drain else 2):
                ex0, chunks0 = stash.pop(0)
                for (cinfo, h2, t2, col2) in chunks0:
                    nc.tensor.matmul(
                        cinfo["tile"][0:W + 1, :],
                        vaug[t2][:, h2, :],
                        ex0[:, col2:col2 + ST],
                        start=(t2 == 0), stop=(t2 == NT - 1),
                        skip_group_check=True,
                    )
                    cinfo["n"] += 1

        def place_private(nslots):
            close_group()
            out = []
            for _ in range(nslots):
                if st_["sc"] is None or st_["used"] >= CH:
                    close_group()
                    new_sc()
                out.append((st_["sc"], st_["used"] * ST))
                st_["used"] += 1
                st_["base"] = st_["used"]
            if st_["used"] >= CH:
                st_["sc"] = None
            return out

        def place_score():
            if st_["sc"] is None or st_["used"] >= CH:
                close_group()
                new_sc()
            sc, col = st_["sc"], st_["used"] * ST
            st_["used"] += 1
            return sc, col

        def note_score(cinfo, h, t):
            st_["chunks"].append((cinfo, h, t))
            if st_["used"] >= CH:
                close_group()

        def pump_epi():
            st_["tick"] += 1
            if pend1 and pend1[0][0]["n"] == NT:
                _, p1, p2 = pend1.pop(0)
                p1()
                pend2.append((st_["tick"] + 2, p2))
            if pend2 and pend2[0][0] <= st_["tick"]:
                pend2.pop(0)[1]()

        def pump():
            emit_stash()
            pump_epi()

        def epilogue(h, si, ctx_t):
            box = {}

            def p1():
                box["sumrow"] = epi.tile([1, ST], F32R, name="sumrow",
                                         tag="sumrow")
                box["ctx_sb"] = epi.tile([W, ST], F32R, name="ctx_sb",
                                         tag="ctx_sb")
                nc.vector.tensor_copy(box["sumrow"][:], ctx_t[W:W + 1, :])
                nc.vector.tensor_copy(box["ctx_sb"][:], ctx_t[0:W, :])

            def p2():
                nc.tensor.matmul(
                    ctx_t[0:W, :], ones_r[:], box["sumrow"][:],
                    start=True, stop=True, skip_group_check=True,
                )
                rc = epi.tile([W, ST], F32, name="rc", tag="rc")
                nc.vector.reciprocal_approx_fast(rc[:], ctx_t[0:W, :])
                ot = epi.tile([W, ST], F32, name="ot", tag="ot")
                nc.vector.tensor_mul(ot[:], box["ctx_sb"][:], rc[:])
                nc.sync.dma_start(
                    out=out_d[h * W:(h + 1) * W, si * ST:(si + 1) * ST],
                    in_=ot[:],
                )
            return p1, p2

        def priv_proj01(si, wch, dk):
            ssl = slice(si * ST, (si + 1) * ST)
            [(sc, col)] = place_private(1)
            for c in range(KC):
                nc.tensor.matmul(
                    sc[:, col:col + ST], wch[c][:, 0:128], xt[c][:, ssl],
                    start=(c == 0), stop=(c == KC - 1), skip_group_check=True,
                )
            dup01(sc[:, col:col + ST], dk, ssl)

        def priv_k(si):
            priv_proj01(si, wk, (kt[0], kt[1]))

        def priv_q01(si):
            priv_proj01(si, wq, (qt[0], qt[1]))

        def priv_b4(si):
            ssl = slice(si * ST, (si + 1) * ST)
            (scA, colA), (scB, colB) = place_private(2)
            pa = scA[:, colA:colA + ST]
            pb = scB[:, colB:colB + ST]
            for c in range(KC):
                stt, spp = (c == 0), (c == KC - 1)
                nc.tensor.matmul(
                    pa[0:64, :], wq[c][0:64, 128:DH], xt[c][0:64, ssl],
                    start=stt, stop=spp, skip_group_check=True,
                )
                nc.tensor.matmul(
                    pb[0:64, :], wq[c][64:128, 128:DH], xt[c][64:128, ssl],
                    start=stt, stop=spp, skip_group_check=True,
                )
                nc.tensor.matmul(
                    pa[64:128, :], wk[c][0:64, 128:DH], xt[c][0:64, ssl],
                    start=stt, stop=spp, skip_group_check=True,
                )
                nc.tensor.matmul(
                    pb[64:128, :], wk[c][64:128, 128:DH], xt[c][64:128, ssl],
                    start=stt, stop=spp, skip_group_check=True,
                )
            th = epi.tile([64, ST], F32, name="b4q", tag="b4q")
            nc.vector.tensor_copy(th[:], pa[0:64, :])
            nc.vector.tensor_add(qt[2][0:64, ssl], th[:], pb[0:64, :])
            nc.vector.tensor_add(qt[2][64:128, ssl], th[:], pb[0:64, :])
            tk = epi.tile([64, ST], F32, name="b4k", tag="b4k")
            nc.vector.tensor_copy(tk[:], pa[64:128, :])
            nc.vector.tensor_add(kt[2][0:64, ssl], tk[:], pb[64:128, :])
            nc.vector.tensor_add(kt[2][64:128, ssl], tk[:], pb[64:128, :])

        def priv_v(t):
            tsl = slice(t * 128, (t + 1) * 128)
            [(sc, col)] = place_private(1)
            pv = sc[:, col:col + DH]
            for c in range(KC):
                nc.tensor.matmul(
                    pv, xt[c][:, tsl], wv[c][:],
                    start=(c == 0), stop=(c == KC - 1), skip_group_check=True,
                )
            nc.vector.tensor_copy(
                vaug[t][:, :, 0:W],
                pv.rearrange("p (h w) -> p h w", h=HL),
            )

        def phase(h, si, privs, nbreak=0):
            qsl = slice(si * ST, (si + 1) * ST)
            cinfo = {"tile": strm.tile([128, ST], F32, name="ctx",
                                       tag="work", bufs=2), "n": 0}
            for j in range(NT // 2):
                for p in privs.get(j, ()):
                    p()
                brk = j >= (NT // 2 - nbreak)
                for half, t in ((0, 2 * j), (1, 2 * j + 1)):
                    sc, col = place_score()
                    rows = slice(0, 64) if half == 0 else slice(64, 128)
                    nc.tensor.matmul(
                        sc[:, col:col + ST],
                        kt[h][rows, t * 128:(t + 1) * 128],
                        qt[h][rows, qsl],
                        start=True, stop=True, skip_group_check=True,
                    )
                    note_score(cinfo, h, t)
                    if brk and half == 0:
                        emit_stash()
                pump()
            p1, p2 = epilogue(h, si, cinfo["tile"])
            pend1.append((cinfo, p1, p2))

        if not has_bias:
            priv_k(0)
            priv_q01(0)
            phase(0, 0, {0: [lambda: priv_v(0), lambda: priv_v(1)],
                         1: [lambda: priv_v(2), lambda: priv_v(3),
                             lambda: priv_k(1)],
                         2: [lambda: priv_v(4), lambda: priv_v(5)],
                         3: [lambda: priv_v(6), lambda: priv_v(7),
                             lambda: priv_k(2)],
                         4: [lambda: priv_v(8), lambda: priv_v(9)],
                         5: [lambda: priv_v(10), lambda: priv_v(11),
                             lambda: priv_k(3)],
                         6: [lambda: priv_v(12), lambda: priv_v(13)],
                         7: [lambda: priv_v(14), lambda: priv_v(15)]})
            phase(0, 1, {0: [lambda: priv_q01(1)]}, nbreak=1)
            phase(0, 2, {0: [lambda: priv_q01(2)]}, nbreak=1)
            phase(0, 3, {0: [lambda: priv_q01(3)]}, nbreak=1)
            phase(1, 0, {0: [lambda: priv_b4(0)]}, nbreak=1)
            phase(1, 1, {0: [lambda: priv_b4(1)]}, nbreak=1)
            phase(1, 2, {0: [lambda: priv_b4(2)]}, nbreak=1)
            phase(1, 3, {0: [lambda: priv_b4(3)]}, nbreak=1)
            for si in range(NS):
                phase(2, si, {}, nbreak=8)
        else:
            for h in range(HL):
                for si in range(NS):
                    phase(h, si, {}, nbreak=8)
        close_group()
        emit_stash(drain=True)
        while pend1:
            _, p1, p2 = pend1.pop(0)
            p1()
            pend2.append((0, p2))
        while pend2:
            pend2.pop(0)[1]()


def _build(has_bias, has_mask):
    nc = bacc.Bacc(
        "TRN2", target_bir_lowering=False, debug=False, num_devices=N_CORES
    )
    xt_d = nc.dram_tensor("xt", [D, S], BF16, kind="ExternalInput").ap()
    wq_d = nc.dram_tensor("wq", [D + 1, DH], BF16, kind="ExternalInput").ap()
    wk_d = nc.dram_tensor("wk", [D + 1, DH], BF16, kind="ExternalInput").ap()
    wv_d = nc.dram_tensor("wv", [D + 1, DH], BF16, kind="ExternalInput").ap()
    on_d = nc.dram_tensor("onesd", [128, PT], BF16, kind="ExternalInput").ap()
    o32_d = nc.dram_tensor("ones32", [1, W], F32R, kind="ExternalInput").ap()
    mb_d = (
        nc.dram_tensor("mb", [128, NT], F32, kind="ExternalInput").ap()
        if has_mask else None
    )
    out_d = nc.dram_tensor("out", [DH, S], F32, kind="ExternalOutput").ap()

    with tile.TileContext(nc) as tc:
        _emit(tc, (xt_d, wq_d, wk_d, wv_d, on_d, o32_d, mb_d, out_d),
              has_bias, has_mask)
    nc.compile()
    return nc


_NC_CACHE = {}


def _get_nc(has_bias, has_mask):
    key = (has_bias, has_mask)
    if key not in _NC_CACHE:
        _NC_CACHE[key] = _build(has_bias, has_mask)
    return _NC_CACHE[key]


def _in_maps(x, Wq, bq, Wk, bk, Wv, bv, mask, has_bias, has_mask):
    xt_by_b = [np.ascontiguousarray(x[b].T).astype(BF) for b in range(B)]
    mb_by_b = [
        np.ascontiguousarray(
            ((np.asarray(mask[b]) == 0).astype(np.float32) * np.float32(-1e30))
            .reshape(NT, 128).T
        )
        for b in range(B)
    ]
    maps = []
    for c in range(N_CORES):
        b, g = divmod(c, N_CORES // B)
        lo = g * DH
        wq_a = np.empty((D + 1, DH), np.float32)
        wq_a[:D] = Wq[lo:lo + DH, :].T
        wq_a[D] = bq[lo:lo + DH]
        wk_a = np.empty((D + 1, DH), np.float32)
        wk_a[:D] = Wk[lo:lo + DH, :].T
        wk_a[D] = bk[lo:lo + DH]
        wv_a = np.empty((D + 1, DH), np.float32)
        wv_a[:D] = Wv[lo:lo + DH, :].T
        wv_a[D] = bv[lo:lo + DH]
        m = {
            "xt": xt_by_b[b], "wq": wq_a.astype(BF), "wk": wk_a.astype(BF),
            "wv": wv_a.astype(BF),
            "onesd": np.ones((128, PT), BF),
            "ones32": _round_f32r(np.ones((1, W), np.float32)),
        }
        if has_mask:
            m["mb"] = mb_by_b[b]
        maps.append(m)
    return maps


def _install_ntff_hook():
    import types

    try:
        from antenv.axon_hooks import get_axon_ntff_profile_hook
        return True
    except ImportError:
        pass
    try:
        import antenv
        from trn_agent_boot.trn_boot import _ntff_profile_via_ctypes

        hook = _ntff_profile_via_ctypes("/opt/axon/libaxon_pjrt.so")
        if hook is None:
            return False
        mod = types.ModuleType("antenv.axon_hooks")
        state = {"hook": hook}
        mod.get_axon_ntff_profile_hook = lambda: state["hook"]
        mod.set_axon_ntff_profile_hook = lambda h: state.update(hook=h)
        sys.modules["antenv.axon_hooks"] = mod
        antenv.axon_hooks = mod
        return True
    except Exception:
        return False


def _run(x, Wq, bq, Wk, bk, Wv, bv, mask, trace=False):
    if trace:
        trace = _install_ntff_hook()
    x = np.ascontiguousarray(np.asarray(x, np.float32))
    Wq = np.asarray(Wq, np.float32)
    Wk = np.asarray(Wk, np.float32)
    Wv = np.asarray(Wv, np.float32)
    bq = np.asarray(bq, np.float32)
    bk = np.asarray(bk, np.float32)
    bv = np.asarray(bv, np.float32)
    has_bias = bool(np.any(bq) or np.any(bk) or np.any(bv))
    has_mask = bool((np.asarray(mask) == 0).any())
    nc = _get_nc(has_bias, has_mask)
    maps = _in_maps(x, Wq, bq, Wk, bk, Wv, bv, mask, has_bias, has_mask)
    res = run_bass_kernel_spmd(nc, maps, list(range(N_CORES)), trace=trace)
    out = np.empty((B, S, D), np.float32)
    for c in range(N_CORES):
        b, g = divmod(c, N_CORES // B)
        out[b, :, g * DH:(g + 1) * DH] = res.results[c]["out"].T
    return out, res


def kernel(x, Wq, bq, Wk, bk, Wv, bv, mask):
    out, _ = _run(x, Wq, bq, Wk, bk, Wv, bv, mask)
    return out


# revision 16
# speedup vs baseline: 1.0216x; 1.0216x over previous
import sys

import ml_dtypes
import numpy as np

_TRN_REPO = "/opt/trn_rl_repo"
if _TRN_REPO not in sys.path:
    sys.path.insert(0, _TRN_REPO)

import concourse.tile as tile
from concourse import bacc, mybir
from concourse.bass_utils import run_bass_kernel_spmd

F32 = mybir.dt.float32
F32R = mybir.dt.float32r
BF16 = mybir.dt.bfloat16
AF = mybir.ActivationFunctionType

B, S, D = 2, 2048, 768
H_TOT, W = 12, 64
N_CORES = 8
HL = 3
DH = HL * W
KC = D // 128
ST = 512
NS = S // ST
PT = 1024
NT = S // 128
BF = ml_dtypes.bfloat16


def _round_f32r(a):
    u = np.ascontiguousarray(a, np.float32).view(np.uint32).copy()
    u += np.uint32(0x7FF) + ((u >> np.uint32(12)) & np.uint32(1))
    u &= np.uint32(0xFFFFF000)
    return u.view(np.float32)


def _emit(tc, aps, has_bias, has_mask):
    nc = tc.nc
    xt_d, wq_d, wk_d, wv_d, on_d, o32_d, mb_d, out_d = aps

    CH = 1 if has_mask else 3
    SCW = CH * ST

    from contextlib import ExitStack

    with ExitStack() as ctx:
        const = ctx.enter_context(tc.tile_pool(name="const", bufs=1))

        ones = None
        if has_bias:
            ones = const.tile([1, PT], BF16, name="ones", tag="ones")
        ones_r = const.tile([1, W], F32R, name="ones_r", tag="ones_r")
        mb = None
        if has_mask:
            mb = const.tile([128, NT], F32, name="mb", tag="mb")

        xt = []
        for c in range(KC):
            t = const.tile([128, S], BF16, name=f"xt{c}", tag=f"xt{c}")
            xt.append(t)

        def w_tiles(name):
            chunks = []
            for c in range(KC):
                t = const.tile([128, DH], BF16, name=f"{name}{c}",
                               tag=f"{name}{c}")
                chunks.append(t)
            brow = const.tile([1, DH], BF16, name=f"{name}b", tag=f"{name}b")
            return chunks, brow

        wq, wqb = w_tiles("wq")
        wk, wkb = w_tiles("wk")
        wv, wvb = w_tiles("wv")

        dmae = [nc.sync, nc.gpsimd]
        for c in range(KC):
            nc.scalar.dma_start(
                out=xt[c][:, 0:ST], in_=xt_d[c * 128:(c + 1) * 128, 0:ST])
            dmae[c % 2].dma_start(
                out=wk[c][:], in_=wk_d[c * 128:(c + 1) * 128, :])
        for c in range(KC):
            dmae[c % 2].dma_start(
                out=wq[c][:], in_=wq_d[c * 128:(c + 1) * 128, :])
        for si in range(1, NS):
            ssl = slice(si * ST, (si + 1) * ST)
            for c in range(KC):
                dmae[(si + c) % 2].dma_start(
                    out=xt[c][:, ssl], in_=xt_d[c * 128:(c + 1) * 128, ssl])
        for c in range(KC):
            dmae[c % 2].dma_start(
                out=wv[c][:], in_=wv_d[c * 128:(c + 1) * 128, :])
        if has_bias:
            for brow, w_d in ((wqb, wq_d), (wkb, wk_d), (wvb, wv_d)):
                nc.sync.dma_start(out=brow[:], in_=w_d[D:D + 1, :])
            nc.sync.dma_start(out=ones[:], in_=on_d[0:1, 0:PT])
        nc.sync.dma_start(out=ones_r[:], in_=o32_d[0:1, :])
        if has_mask:
            nc.sync.dma_start(out=mb[:], in_=mb_d[:, :])

        kt01 = const.tile([128, S], BF16, name="kt01", tag="kt01")
        qt01 = const.tile([128, S], BF16, name="qt01", tag="qt01")
        kt2 = const.tile([64, S], BF16, name="kt2", tag="kt2")
        qt2 = const.tile([64, S], BF16, name="qt2", tag="qt2")
        vaug = []
        for t in range(NT):
            va = const.tile([128, HL, W + 1], BF16, name=f"vaug{t}",
                            tag=f"vaug{t}")
            nc.gpsimd.memset(va[:, :, W:W + 1], 1.0)
            vaug.append(va)

        strm = ctx.enter_context(
            tc.tile_pool(name="strm", bufs=1, space="PSUM"))
        epi = ctx.enter_context(tc.tile_pool(name="epi", bufs=2))
        exp_pool = ctx.enter_context(tc.tile_pool(name="exp", bufs=6))

        def proj_main(si, dst, brow, wch):
            ssl = slice(si * ST, (si + 1) * ST)
            wrk = strm.tile([128, ST], F32, name="wrk", tag="work", bufs=2)
            for c in range(KC):
                nc.tensor.matmul(
                    wrk[:], wch[c][:, 0:128], xt[c][:, ssl],
                    start=(c == 0), stop=False, skip_group_check=True,
                )
            nc.tensor.matmul(
                wrk[:], brow[:, 0:128], ones[:, 0:ST],
                start=False, stop=True, skip_group_check=True,
            )
            nc.vector.tensor_copy(dst[:, ssl], wrk[:])

        def proj_h2_bias(si, wch, brow, dst):
            ssl = slice(si * ST, (si + 1) * ST)
            wrk = strm.tile([128, ST], F32, name="wrkb", tag="work", bufs=2)
            for c in range(KC):
                nc.tensor.matmul(
                    wrk[0:64, :], wch[c][:, 128:DH], xt[c][:, ssl],
                    start=(c == 0), stop=False, skip_group_check=True,
                )
            nc.tensor.matmul(
                wrk[0:64, :], brow[:, 128:DH], ones[:, 0:ST],
                start=False, stop=True, skip_group_check=True,
            )
            nc.vector.tensor_copy(dst[0:64, ssl], wrk[0:64, :])

        if has_bias:
            for si in range(NS):
                proj_main(si, kt01, wkb, wk)
                proj_h2_bias(si, wk, wkb, kt2)
                proj_main(si, qt01, wqb, wq)
                proj_h2_bias(si, wq, wqb, qt2)
            for t in range(NT):
                tsl = slice(t * 128, (t + 1) * 128)
                wrk = strm.tile([128, ST], F32, name="wrkv", tag="work",
                                bufs=2)
                for c in range(KC):
                    nc.tensor.matmul(
                        wrk[:, 0:DH], xt[c][:, tsl], wv[c][:],
                        start=(c == 0), stop=False, skip_group_check=True,
                    )
                nc.tensor.matmul(
                    wrk[:, 0:DH], ones[:, 0:128], wvb[:],
                    start=False, stop=True, skip_group_check=True,
                )
                nc.vector.tensor_copy(
                    vaug[t][:, :, 0:W],
                    wrk[:, 0:DH].rearrange("p (h w) -> p h w", h=HL),
                )

        st_ = {"sc": None, "used": 0, "base": 0, "chunks": [], "tick": 0}
        stash = []
        pend1 = []
        pend2 = []

        def new_sc():
            st_["sc"] = strm.tile([128, SCW], F32, name="sc", tag="sc",
                                  bufs=2)
            st_["used"] = 0
            st_["base"] = 0
            st_["chunks"] = []

        def close_group():
            sc = st_["sc"]
            if sc is None:
                return
            n = len(st_["chunks"])
            if n:
                lo = st_["base"] * ST
                hi = lo + n * ST
                ex = exp_pool.tile([128, SCW], BF16, name="ex", tag="ex")
                tlast = st_["chunks"][-1][2]
                nc.scalar.activation(
                    ex[:, 0:n * ST], sc[:, lo:hi], AF.Exp,
                    bias=(mb[:, tlast:tlast + 1] if has_mask else 0.0),
                    scale=0.125,
                )
                stash.append(
                    (ex, [(cinfo, h, t, i * ST)
                          for i, (cinfo, h, t) in enumerate(st_["chunks"])]))
            st_["sc"] = None

        def emit_stash(drain=False):
            while len(stash) > (0 if drain else 2):
                ex0, chunks0 = stash.pop(0)
                for (cinfo, h2, t2, col2) in chunks0:
                    nc.tensor.matmul(
                        cinfo["tile"][0:W + 1, :],
                        vaug[t2][:, h2, :],
                        ex0[:, col2:col2 + ST],
                        start=(t2 == 0), stop=(t2 == NT - 1),
                        skip_group_check=True,
                    )
                    cinfo["n"] += 1

        def place_private(nslots):
            close_group()
            out = []
            for _ in range(nslots):
                if st_["sc"] is None or st_["used"] >= CH:
                    close_group()
                    new_sc()
                out.append((st_["sc"], st_["used"] * ST))
                st_["used"] += 1
                st_["base"] = st_["used"]
            if st_["used"] >= CH:
                st_["sc"] = None
            return out

        def place_score():
            if st_["sc"] is None or st_["used"] >= CH:
                close_group()
                new_sc()
            sc, col = st_["sc"], st_["used"] * ST
            st_["used"] += 1
            return sc, col

        def note_score(cinfo, h, t):
            st_["chunks"].append((cinfo, h, t))
            if st_["used"] >= CH:
                close_group()

        def pump_epi():
            st_["tick"] += 1
            if pend1 and pend1[0][0]["n"] == NT:
                _, p1, p2 = pend1.pop(0)
                p1()
                pend2.append((st_["tick"] + 4, p2))
            if pend2 and pend2[0][0] <= st_["tick"]:
                pend2.pop(0)[1]()

        def pump():
            emit_stash()
            pump_epi()

        def epilogue(h, si, ctx_t):
            box = {}

            def p1():
                box["sumrow"] = epi.tile([1, ST], F32R, name="sumrow",
                                         tag="sumrow")
                box["ctx_sb"] = epi.tile([W, ST], F32R, name="ctx_sb",
                                         tag="ctx_sb")
                nc.vector.tensor_copy(box["sumrow"][:], ctx_t[W:W + 1, :])
                nc.vector.tensor_copy(box["ctx_sb"][:], ctx_t[0:W, :])

            def p2():
                nc.tensor.matmul(
                    ctx_t[0:W, :], ones_r[:], box["sumrow"][:],
                    start=True, stop=True, skip_group_check=True,
                )
                rc = epi.tile([W, ST], F32, name="rc", tag="rc")
                nc.vector.reciprocal_approx_fast(rc[:], ctx_t[0:W, :])
                ot = epi.tile([W, ST], F32, name="ot", tag="ot")
                nc.vector.tensor_mul(ot[:], box["ctx_sb"][:], rc[:])
                nc.sync.dma_start(
                    out=out_d[h * W:(h + 1) * W, si * ST:(si + 1) * ST],
                    in_=ot[:],
                )
            return p1, p2

        def priv_proj01(si, wch, dst):
            ssl = slice(si * ST, (si + 1) * ST)
            [(sc, col)] = place_private(1)
            for c in range(KC):
                nc.tensor.matmul(
                    sc[:, col:col + ST], wch[c][:, 0:128], xt[c][:, ssl],
                    start=(c == 0), stop=(c == KC - 1), skip_group_check=True,
                )
            nc.vector.tensor_copy(dst[:, ssl], sc[:, col:col + ST])

        def priv_k(si):
            priv_proj01(si, wk, kt01)

        def priv_q01(si):
            priv_proj01(si, wq, qt01)

        def priv_b4(si):
            ssl = slice(si * ST, (si + 1) * ST)
            (scA, colA), (scB, colB) = place_private(2)
            pa = scA[:, colA:colA + ST]
            pb = scB[:, colB:colB + ST]
            for c in range(KC):
                stt, spp = (c == 0), (c == KC - 1)
                nc.tensor.matmul(
                    pa[0:64, :], wq[c][0:64, 128:DH], xt[c][0:64, ssl],
                    start=stt, stop=spp, skip_group_check=True,
                )
                nc.tensor.matmul(
                    pb[0:64, :], wq[c][64:128, 128:DH], xt[c][64:128, ssl],
                    start=stt, stop=spp, skip_group_check=True,
                )
                nc.tensor.matmul(
                    pa[64:128, :], wk[c][0:64, 128:DH], xt[c][0:64, ssl],
                    start=stt, stop=spp, skip_group_check=True,
                )
                nc.tensor.matmul(
                    pb[64:128, :], wk[c][64:128, 128:DH], xt[c][64:128, ssl],
                    start=stt, stop=spp, skip_group_check=True,
                )
            th = epi.tile([64, ST], F32, name="b4q", tag="b4q")
            nc.vector.tensor_copy(th[:], pa[0:64, :])
            nc.vector.tensor_add(qt2[:, ssl], th[:], pb[0:64, :])
            tk = epi.tile([64, ST], F32, name="b4k", tag="b4k")
            nc.vector.tensor_copy(tk[:], pa[64:128, :])
            nc.vector.tensor_add(kt2[:, ssl], tk[:], pb[64:128, :])

        def priv_v(t):
            tsl = slice(t * 128, (t + 1) * 128)
            [(sc, col)] = place_private(1)
            pv = sc[:, col:col + DH]
            for c in range(KC):
                nc.tensor.matmul(
                    pv, xt[c][:, tsl], wv[c][:],
                    start=(c == 0), stop=(c == KC - 1), skip_group_check=True,
                )
            nc.vector.tensor_copy(
                vaug[t][:, :, 0:W],
                pv.rearrange("p (h w) -> p h w", h=HL),
            )

        def phase(h, si, privs):
            qsl = slice(si * ST, (si + 1) * ST)
            if h == 0:
                ktile, qtile, rows = kt01, qt01, slice(0, 64)
            elif h == 1:
                ktile, qtile, rows = kt01, qt01, slice(64, 128)
            else:
                ktile, qtile, rows = kt2, qt2, slice(0, 64)
            cinfo = {"tile": strm.tile([128, ST], F32, name="ctx",
                                       tag="work", bufs=2), "n": 0}
            for t in range(NT):
                for p in privs.get(t, ()):
                    p()
                sc, col = place_score()
                nc.tensor.matmul(
                    sc[:, col:col + ST],
                    ktile[rows, t * 128:(t + 1) * 128],
                    qtile[rows, qsl],
                    start=True, stop=True, skip_group_check=True,
                )
                note_score(cinfo, h, t)
                pump()
            p1, p2 = epilogue(h, si, cinfo["tile"])
            pend1.append((cinfo, p1, p2))

        if not has_bias:
            priv_k(0)
            priv_q01(0)
            phase(0, 0, {2: [lambda: priv_v(0), lambda: priv_v(1)],
                         3: [lambda: priv_v(2), lambda: priv_v(3),
                             lambda: priv_k(1)],
                         4: [lambda: priv_v(4), lambda: priv_v(5)],
                         6: [lambda: priv_v(6), lambda: priv_v(7),
                             lambda: priv_k(2)],
                         8: [lambda: priv_v(8), lambda: priv_v(9)],
                         10: [lambda: priv_v(10), lambda: priv_v(11),
                              lambda: priv_k(3)],
                         11: [lambda: priv_v(12), lambda: priv_v(13)],
                         12: [lambda: priv_v(14), lambda: priv_v(15)],
                         13: [lambda: priv_q01(1)]})
            phase(0, 1, {4: [lambda: priv_b4(0)],
                         12: [lambda: priv_q01(2)]})
            phase(0, 2, {4: [lambda: priv_b4(1)],
                         12: [lambda: priv_q01(3)]})
            phase(0, 3, {4: [lambda: priv_b4(2)]})
            phase(1, 0, {4: [lambda: priv_b4(3)]})
            for si in range(1, NS):
                phase(1, si, {})
            for si in range(NS):
                phase(2, si, {})
        else:
            for h in range(HL):
                for si in range(NS):
                    phase(h, si, {})
        close_group()
        emit_stash(drain=True)
        while pend1:
            _, p1, p2 = pend1.pop(0)
            p1()
            pend2.append((0, p2))
        while pend2:
            pend2.pop(0)[1]()


def _build(has_bias, has_mask):
    nc = bacc.Bacc(
        "TRN2", target_bir_lowering=False, debug=False, num_devices=N_CORES
    )
    xt_d = nc.dram_tensor("xt", [D, S], BF16, kind="ExternalInput").ap()
    wq_d = nc.dram_tensor("wq", [D + 1, DH], BF16, kind="ExternalInput").ap()
    wk_d = nc.dram_tensor("wk", [D + 1, DH], BF16, kind="ExternalInput").ap()
    wv_d = nc.dram_tensor("wv", [D + 1, DH], BF16, kind="ExternalInput").ap()
    on_d = nc.dram_tensor("onesd", [128, PT], BF16, kind="ExternalInput").ap()
    o32_d = nc.dram_tensor("ones32", [1, W], F32R, kind="ExternalInput").ap()
    mb_d = (
        nc.dram_tensor("mb", [128, NT], F32, kind="ExternalInput").ap()
        if has_mask else None
    )
    out_d = nc.dram_tensor("out", [DH, S], F32, kind="ExternalOutput").ap()

    with tile.TileContext(nc) as tc:
        _emit(tc, (xt_d, wq_d, wk_d, wv_d, on_d, o32_d, mb_d, out_d),
              has_bias, has_mask)
    nc.compile()
    return nc


_NC_CACHE = {}


def _get_nc(has_bias, has_mask):
    key = (has_bias, has_mask)
    if key not in _NC_CACHE:
        _NC_CACHE[key] = _build(has_bias, has_mask)
    return _NC_CACHE[key]


def _in_maps(x, Wq, bq, Wk, bk, Wv, bv, mask, has_bias, has_mask):
    xt_by_b = [np.ascontiguousarray(x[b].T).astype(BF) for b in range(B)]
    mb_by_b = [
        np.ascontiguousarray(
            ((np.asarray(mask[b]) == 0).astype(np.float32) * np.float32(-1e30))
            .reshape(NT, 128).T
        )
        for b in range(B)
    ]
    maps = []
    for c in range(N_CORES):
        b, g = divmod(c, N_CORES // B)
        lo = g * DH
        wq_a = np.empty((D + 1, DH), np.float32)
        wq_a[:D] = Wq[lo:lo + DH, :].T
        wq_a[D] = bq[lo:lo + DH]
        wk_a = np.empty((D + 1, DH), np.float32)
        wk_a[:D] = Wk[lo:lo + DH, :].T
        wk_a[D] = bk[lo:lo + DH]
        wv_a = np.empty((D + 1, DH), np.float32)
        wv_a[:D] = Wv[lo:lo + DH, :].T
        wv_a[D] = bv[lo:lo + DH]
        m = {
            "xt": xt_by_b[b], "wq": wq_a.astype(BF), "wk": wk_a.astype(BF),
            "wv": wv_a.astype(BF),
            "onesd": np.ones((128, PT), BF),
            "ones32": _round_f32r(np.ones((1, W), np.float32)),
        }
        if has_mask:
            m["mb"] = mb_by_b[b]
        maps.append(m)
    return maps


def _install_ntff_hook():
    import types

    try:
        from antenv.axon_hooks import get_axon_ntff_profile_hook
        return True
    except ImportError:
        pass
    try:
        import antenv
        from trn_agent_boot.trn_boot import _ntff_profile_via_ctypes

        hook = _ntff_profile_via_ctypes("/opt/axon/libaxon_pjrt.so")
        if hook is None:
            return False
        mod = types.ModuleType("antenv.axon_hooks")
        state = {"hook": hook}
        mod.get_axon_ntff_profile_hook = lambda: state["hook"]
        mod.set_axon_ntff_profile_hook = lambda h: state.update(hook=h)
        sys.modules["antenv.axon_hooks"] = mod
        antenv.axon_hooks = mod
        return True
    except Exception:
        return False


def _run(x, Wq, bq, Wk, bk, Wv, bv, mask, trace=False):
    if trace:
        trace = _install_ntff_hook()
    x = np.ascontiguousarray(np.asarray(x, np.float32))
    Wq = np.asarray(Wq, np.float32)
    Wk = np.asarray(Wk, np.float32)
    Wv = np.asarray(Wv, np.float32)
    bq = np.asarray(bq, np.float32)
    bk = np.asarray(bk, np.float32)
    bv = np.asarray(bv, np.float32)
    has_bias = bool(np.any(bq) or np.any(bk) or np.any(bv))
    has_mask = bool((np.asarray(mask) == 0).any())
    nc = _get_nc(has_bias, has_mask)
    maps = _in_maps(x, Wq, bq, Wk, bk, Wv, bv, mask, has_bias, has_mask)
    res = run_bass_kernel_spmd(nc, maps, list(range(N_CORES)), trace=trace)
    out = np.empty((B, S, D), np.float32)
    for c in range(N_CORES):
        b, g = divmod(c, N_CORES // B)
        out[b, :, g * DH:(g + 1) * DH] = res.results[c]["out"].T
    return out, res


def kernel(x, Wq, bq, Wk, bk, Wv, bv, mask):
    out, _ = _run(x, Wq, bq, Wk, bk, Wv, bv, mask)
    return out


# revision 19
# speedup vs baseline: 1.1417x; 1.1176x over previous
import sys

import ml_dtypes
import numpy as np

_TRN_REPO = "/opt/trn_rl_repo"
if _TRN_REPO not in sys.path:
    sys.path.insert(0, _TRN_REPO)

import concourse.tile as tile
from concourse import bacc, mybir
from concourse.bass_utils import run_bass_kernel_spmd

F32 = mybir.dt.float32
F32R = mybir.dt.float32r
BF16 = mybir.dt.bfloat16
AF = mybir.ActivationFunctionType

B, S, D = 2, 2048, 768
H_TOT, W = 12, 64
N_CORES = 8
HL = 3
DH = HL * W
KC = D // 128
ST = 512
NS = S // ST
PT = 1024
NT = S // 128
BF = ml_dtypes.bfloat16


def _round_f32r(a):
    u = np.ascontiguousarray(a, np.float32).view(np.uint32).copy()
    u += np.uint32(0x7FF) + ((u >> np.uint32(12)) & np.uint32(1))
    u &= np.uint32(0xFFFFF000)
    return u.view(np.float32)


def _emit(tc, aps, has_bias, has_mask):
    nc = tc.nc
    xt_d, wq_d, wk_d, wv_d, on_d, o32_d, mb_d, out_d = aps

    CH = 1 if has_mask else 3
    SCW = CH * ST

    from contextlib import ExitStack

    with ExitStack() as ctx:
        const = ctx.enter_context(tc.tile_pool(name="const", bufs=1))

        ones = None
        if has_bias:
            ones = const.tile([1, PT], BF16, name="ones", tag="ones")
        ones_r = const.tile([1, W], F32R, name="ones_r", tag="ones_r")
        mb = None
        if has_mask:
            mb = const.tile([128, NT], F32, name="mb", tag="mb")

        xt = []
        for c in range(KC):
            t = const.tile([128, S], BF16, name=f"xt{c}", tag=f"xt{c}")
            xt.append(t)

        def w_tiles(name):
            chunks = []
            for c in range(KC):
                t = const.tile([128, DH], BF16, name=f"{name}{c}",
                               tag=f"{name}{c}")
                chunks.append(t)
            brow = const.tile([1, DH], BF16, name=f"{name}b", tag=f"{name}b")
            return chunks, brow

        wq, wqb = w_tiles("wq")
        wk, wkb = w_tiles("wk")
        wv, wvb = w_tiles("wv")

        dmae = [nc.sync, nc.gpsimd]
        for c in range(KC):
            nc.scalar.dma_start(
                out=xt[c][:, 0:ST], in_=xt_d[c * 128:(c + 1) * 128, 0:ST])
            dmae[c % 2].dma_start(
                out=wk[c][:], in_=wk_d[c * 128:(c + 1) * 128, :])
        for c in range(KC):
            dmae[c % 2].dma_start(
                out=wq[c][:], in_=wq_d[c * 128:(c + 1) * 128, :])
        for si in range(1, NS):
            ssl = slice(si * ST, (si + 1) * ST)
            for c in range(KC):
                dmae[(si + c) % 2].dma_start(
                    out=xt[c][:, ssl], in_=xt_d[c * 128:(c + 1) * 128, ssl])
        for c in range(KC):
            dmae[c % 2].dma_start(
                out=wv[c][:], in_=wv_d[c * 128:(c + 1) * 128, :])
        if has_bias:
            for brow, w_d in ((wqb, wq_d), (wkb, wk_d), (wvb, wv_d)):
                nc.sync.dma_start(out=brow[:], in_=w_d[D:D + 1, :])
            nc.sync.dma_start(out=ones[:], in_=on_d[0:1, 0:PT])
        nc.sync.dma_start(out=ones_r[:], in_=o32_d[0:1, :])
        if has_mask:
            nc.sync.dma_start(out=mb[:], in_=mb_d[:, :])

        kt01 = const.tile([128, S], BF16, name="kt01", tag="kt01")
        qt01 = const.tile([128, S], BF16, name="qt01", tag="qt01")
        kt2 = const.tile([64, S], BF16, name="kt2", tag="kt2")
        qt2 = const.tile([64, S], BF16, name="qt2", tag="qt2")
        vaug = []
        for t in range(NT):
            va = const.tile([128, HL, W + 1], BF16, name=f"vaug{t}",
                            tag=f"vaug{t}")
            nc.gpsimd.memset(va[:, :, W:W + 1], 1.0)
            vaug.append(va)

        strm = ctx.enter_context(
            tc.tile_pool(name="strm", bufs=1, space="PSUM"))
        epi = ctx.enter_context(tc.tile_pool(name="epi", bufs=2))
        exp_pool = ctx.enter_context(tc.tile_pool(name="exp", bufs=6))

        def proj_main(si, dst, brow, wch):
            ssl = slice(si * ST, (si + 1) * ST)
            wrk = strm.tile([128, ST], F32, name="wrk", tag="work", bufs=2)
            for c in range(KC):
                nc.tensor.matmul(
                    wrk[:], wch[c][:, 0:128], xt[c][:, ssl],
                    start=(c == 0), stop=False, skip_group_check=True,
                )
            nc.tensor.matmul(
                wrk[:], brow[:, 0:128], ones[:, 0:ST],
                start=False, stop=True, skip_group_check=True,
            )
            nc.vector.tensor_copy(dst[:, ssl], wrk[:])

        def proj_h2_bias(si, wch, brow, dst):
            ssl = slice(si * ST, (si + 1) * ST)
            wrk = strm.tile([128, ST], F32, name="wrkb", tag="work", bufs=2)
            for c in range(KC):
                nc.tensor.matmul(
                    wrk[0:64, :], wch[c][:, 128:DH], xt[c][:, ssl],
                    start=(c == 0), stop=False, skip_group_check=True,
                )
            nc.tensor.matmul(
                wrk[0:64, :], brow[:, 128:DH], ones[:, 0:ST],
                start=False, stop=True, skip_group_check=True,
            )
            nc.vector.tensor_copy(dst[0:64, ssl], wrk[0:64, :])

        if has_bias:
            for si in range(NS):
                proj_main(si, kt01, wkb, wk)
                proj_h2_bias(si, wk, wkb, kt2)
                proj_main(si, qt01, wqb, wq)
                proj_h2_bias(si, wq, wqb, qt2)
            for t in range(NT):
                tsl = slice(t * 128, (t + 1) * 128)
                wrk = strm.tile([128, ST], F32, name="wrkv", tag="work",
                                bufs=2)
                for c in range(KC):
                    nc.tensor.matmul(
                        wrk[:, 0:DH], xt[c][:, tsl], wv[c][:],
                        start=(c == 0), stop=False, skip_group_check=True,
                    )
                nc.tensor.matmul(
                    wrk[:, 0:DH], ones[:, 0:128], wvb[:],
                    start=False, stop=True, skip_group_check=True,
                )
                nc.vector.tensor_copy(
                    vaug[t][:, :, 0:W],
                    wrk[:, 0:DH].rearrange("p (h w) -> p h w", h=HL),
                )

        st_ = {"sc": None, "used": 0, "base": 0, "chunks": [], "tick": 0,
               "keep": 2, "fill": 0}
        stash = []
        pend1 = []
        pend2 = []

        def new_sc():
            st_["sc"] = strm.tile([128, SCW], F32, name="sc", tag="sc",
                                  bufs=2)
            st_["used"] = 0
            st_["base"] = 0
            st_["chunks"] = []

        def close_group():
            sc = st_["sc"]
            if sc is None:
                return
            n = len(st_["chunks"])
            if n:
                lo = st_["base"] * ST
                hi = lo + n * ST
                ex = exp_pool.tile([128, SCW], BF16, name="ex", tag="ex")
                tlast = st_["chunks"][-1][2]
                nc.scalar.activation(
                    ex[:, 0:n * ST], sc[:, lo:hi], AF.Exp,
                    bias=(mb[:, tlast:tlast + 1] if has_mask else 0.0),
                    scale=0.125,
                )
                stash.append(
                    (ex, [(cinfo, h, t, i * ST)
                          for i, (cinfo, h, t) in enumerate(st_["chunks"])]))
            st_["sc"] = None

        def emit_stash(drain=False):
            n_pop = len(stash) if drain else (
                1 if len(stash) > st_["keep"] else 0)
            for _ in range(n_pop):
                ex0, chunks0 = stash.pop(0)
                for (cinfo, h2, t2, col2) in chunks0:
                    nc.tensor.matmul(
                        cinfo["tile"][0:W + 1, :],
                        vaug[t2][:, h2, :],
                        ex0[:, col2:col2 + ST],
                        start=(t2 == 0), stop=(t2 == NT - 1),
                        skip_group_check=True,
                    )
                    cinfo["n"] += 1

        def place_private(nslots):
            close_group()
            out = []
            for _ in range(nslots):
                if st_["sc"] is None or st_["used"] >= CH:
                    close_group()
                    new_sc()
                out.append((st_["sc"], st_["used"] * ST))
                st_["used"] += 1
                st_["base"] = st_["used"]
            if st_["used"] >= CH:
                st_["sc"] = None
            return out

        def place_score():
            if st_["sc"] is None or st_["used"] >= CH:
                close_group()
                new_sc()
            sc, col = st_["sc"], st_["used"] * ST
            st_["used"] += 1
            return sc, col

        def note_score(cinfo, h, t):
            st_["chunks"].append((cinfo, h, t))
            if st_["used"] >= CH:
                close_group()

        def pump_epi():
            st_["tick"] += 1
            if pend1 and pend1[0][0]["n"] == NT:
                _, p1, p2 = pend1.pop(0)
                p1()
                pend2.append((st_["tick"] + 4, p2))
            if pend2 and pend2[0][0] <= st_["tick"]:
                pend2.pop(0)[1]()

        def pump(ctx_t=None):
            emit_stash()
            pump_epi()
            if st_["fill"] and ctx_t is not None:
                nc.tensor.matmul(
                    ctx_t[96:128, 0:st_["fill"]],
                    kt01[0:64, 0:32], qt01[0:64, 0:st_["fill"]],
                    start=True, stop=True, skip_group_check=True,
                    tile_position=(0, 96),
                )

        def epilogue(h, si, ctx_t):
            box = {}

            def p1():
                box["sumrow"] = epi.tile([1, ST], F32R, name="sumrow",
                                         tag="sumrow")
                box["ctx_sb"] = epi.tile([W, ST], F32R, name="ctx_sb",
                                         tag="ctx_sb")
                nc.vector.tensor_copy(box["sumrow"][:], ctx_t[W:W + 1, :])
                nc.vector.tensor_copy(box["ctx_sb"][:], ctx_t[0:W, :])

            def p2():
                nc.tensor.matmul(
                    ctx_t[0:W, :], ones_r[:], box["sumrow"][:],
                    start=True, stop=True, skip_group_check=True,
                )
                rc = epi.tile([W, ST], F32, name="rc", tag="rc")
                nc.vector.reciprocal_approx_fast(rc[:], ctx_t[0:W, :])
                ot = epi.tile([W, ST], F32, name="ot", tag="ot")
                nc.vector.tensor_mul(ot[:], box["ctx_sb"][:], rc[:])
                nc.sync.dma_start(
                    out=out_d[h * W:(h + 1) * W, si * ST:(si + 1) * ST],
                    in_=ot[:],
                )
            return p1, p2

        def priv_proj01(si, wch, dst):
            ssl = slice(si * ST, (si + 1) * ST)
            [(sc, col)] = place_private(1)
            for c in range(KC):
                nc.tensor.matmul(
                    sc[:, col:col + ST], wch[c][:, 0:128], xt[c][:, ssl],
                    start=(c == 0), stop=(c == KC - 1), skip_group_check=True,
                )
            nc.vector.tensor_copy(dst[:, ssl], sc[:, col:col + ST])

        def priv_k(si):
            priv_proj01(si, wk, kt01)

        def priv_q01(si):
            priv_proj01(si, wq, qt01)

        def priv_b4(si):
            ssl = slice(si * ST, (si + 1) * ST)
            (scA, colA), (scB, colB) = place_private(2)
            pa = scA[:, colA:colA + ST]
            pb = scB[:, colB:colB + ST]
            for c in range(KC):
                stt, spp = (c == 0), (c == KC - 1)
                nc.tensor.matmul(
                    pa[0:64, :], wq[c][0:64, 128:DH], xt[c][0:64, ssl],
                    start=stt, stop=spp, skip_group_check=True,
                )
                nc.tensor.matmul(
                    pb[0:64, :], wq[c][64:128, 128:DH], xt[c][64:128, ssl],
                    start=stt, stop=spp, skip_group_check=True,
                )
                nc.tensor.matmul(
                    pa[64:128, :], wk[c][0:64, 128:DH], xt[c][0:64, ssl],
                    start=stt, stop=spp, skip_group_check=True,
                )
                nc.tensor.matmul(
                    pb[64:128, :], wk[c][64:128, 128:DH], xt[c][64:128, ssl],
                    start=stt, stop=spp, skip_group_check=True,
                )
            th = epi.tile([64, ST], F32, name="b4q", tag="b4q")
            nc.vector.tensor_copy(th[:], pa[0:64, :])
            nc.vector.tensor_add(qt2[:, ssl], th[:], pb[0:64, :])
            tk = epi.tile([64, ST], F32, name="b4k", tag="b4k")
            nc.vector.tensor_copy(tk[:], pa[64:128, :])
            nc.vector.tensor_add(kt2[:, ssl], tk[:], pb[64:128, :])

        def priv_v(t):
            tsl = slice(t * 128, (t + 1) * 128)
            [(sc, col)] = place_private(1)
            pv = sc[:, col:col + DH]
            for c in range(KC):
                nc.tensor.matmul(
                    pv, xt[c][:, tsl], wv[c][:],
                    start=(c == 0), stop=(c == KC - 1), skip_group_check=True,
                )
            nc.vector.tensor_copy(
                vaug[t][:, :, 0:W],
                pv.rearrange("p (h w) -> p h w", h=HL),
            )

        def phase(h, si, privs, keep=2, fill=0):
            st_["keep"] = keep
            st_["fill"] = fill
            qsl = slice(si * ST, (si + 1) * ST)
            if h == 0:
                ktile, qtile, rows = kt01, qt01, slice(0, 64)
            elif h == 1:
                ktile, qtile, rows = kt01, qt01, slice(64, 128)
            else:
                ktile, qtile, rows = kt2, qt2, slice(0, 64)
            cinfo = {"tile": strm.tile([128, ST], F32, name="ctx",
                                       tag="work", bufs=2), "n": 0}
            for t in range(NT):
                for p in privs.get(t, ()):
                    p()
                sc, col = place_score()
                nc.tensor.matmul(
                    sc[:, col:col + ST],
                    ktile[rows, t * 128:(t + 1) * 128],
                    qtile[rows, qsl],
                    start=True, stop=True, skip_group_check=True,
                )
                note_score(cinfo, h, t)
                pump(cinfo["tile"])
            p1, p2 = epilogue(h, si, cinfo["tile"])
            pend1.append((cinfo, p1, p2))

        if not has_bias:
            priv_k(0)
            priv_q01(0)
            phase(0, 0, {2: [lambda: priv_v(0), lambda: priv_v(1)],
                         4: [lambda: priv_v(2), lambda: priv_v(3),
                             lambda: priv_k(1)],
                         6: [lambda: priv_v(4), lambda: priv_v(5)],
                         8: [lambda: priv_v(6), lambda: priv_v(7),
                             lambda: priv_k(2)],
                         10: [lambda: priv_k(3)],
                         13: [lambda: priv_q01(1)]},
                  keep=8)
            phase(0, 1, {0: [lambda: priv_v(8), lambda: priv_v(9)],
                         1: [lambda: priv_v(10), lambda: priv_v(11)],
                         2: [lambda: priv_v(12), lambda: priv_v(13)],
                         3: [lambda: priv_v(14), lambda: priv_v(15)],
                         12: [lambda: priv_q01(2)]})
            phase(0, 2, {12: [lambda: priv_q01(3)]})
            phase(0, 3, {}, fill=192)
            phase(1, 0, {4: [lambda: priv_b4(0)]})
            phase(1, 1, {4: [lambda: priv_b4(1)]})
            phase(1, 2, {4: [lambda: priv_b4(2)]})
            phase(1, 3, {4: [lambda: priv_b4(3)]})
            for si in range(NS):
                phase(2, si, {}, fill=192)
        else:
            for h in range(HL):
                for si in range(NS):
                    phase(h, si, {})
        close_group()
        emit_stash(drain=True)
        while pend1:
            _, p1, p2 = pend1.pop(0)
            p1()
            pend2.append((0, p2))
        while pend2:
            pend2.pop(0)[1]()


def _build(has_bias, has_mask):
    nc = bacc.Bacc(
        "TRN2", target_bir_lowering=False, debug=False, num_devices=N_CORES
    )
    xt_d = nc.dram_tensor("xt", [D, S], BF16, kind="ExternalInput").ap()
    wq_d = nc.dram_tensor("wq", [D + 1, DH], BF16, kind="ExternalInput").ap()
    wk_d = nc.dram_tensor("wk", [D + 1, DH], BF16, kind="ExternalInput").ap()
    wv_d = nc.dram_tensor("wv", [D + 1, DH], BF16, kind="ExternalInput").ap()
    on_d = nc.dram_tensor("onesd", [128, PT], BF16, kind="ExternalInput").ap()
    o32_d = nc.dram_tensor("ones32", [1, W], F32R, kind="ExternalInput").ap()
    mb_d = (
        nc.dram_tensor("mb", [128, NT], F32, kind="ExternalInput").ap()
        if has_mask else None
    )
    out_d = nc.dram_tensor("out", [DH, S], F32, kind="ExternalOutput").ap()

    with tile.TileContext(nc) as tc:
        _emit(tc, (xt_d, wq_d, wk_d, wv_d, on_d, o32_d, mb_d, out_d),
              has_bias, has_mask)
    nc.compile()
    return nc


_NC_CACHE = {}


def _get_nc(has_bias, has_mask):
    key = (has_bias, has_mask)
    if key not in _NC_CACHE:
        _NC_CACHE[key] = _build(has_bias, has_mask)
    return _NC_CACHE[key]


def _in_maps(x, Wq, bq, Wk, bk, Wv, bv, mask, has_bias, has_mask):
    xt_by_b = [np.ascontiguousarray(x[b].T).astype(BF) for b in range(B)]
    mb_by_b = [
        np.ascontiguousarray(
            ((np.asarray(mask[b]) == 0).astype(np.float32) * np.float32(-1e30))
            .reshape(NT, 128).T
        )
        for b in range(B)
    ]
    maps = []
    for c in range(N_CORES):
        b, g = divmod(c, N_CORES // B)
        lo = g * DH
        wq_a = np.empty((D + 1, DH), np.float32)
        wq_a[:D] = Wq[lo:lo + DH, :].T
        wq_a[D] = bq[lo:lo + DH]
        wk_a = np.empty((D + 1, DH), np.float32)
        wk_a[:D] = Wk[lo:lo + DH, :].T
        wk_a[D] = bk[lo:lo + DH]
        wv_a = np.empty((D + 1, DH), np.float32)
        wv_a[:D] = Wv[lo:lo + DH, :].T
        wv_a[D] = bv[lo:lo + DH]
        m = {
            "xt": xt_by_b[b], "wq": wq_a.astype(BF), "wk": wk_a.astype(BF),
            "wv": wv_a.astype(BF),
            "onesd": np.ones((128, PT), BF),
            "ones32": _round_f32r(np.ones((1, W), np.float32)),
        }
        if has_mask:
            m["mb"] = mb_by_b[b]
        maps.append(m)
    return maps


def _install_ntff_hook():
    import types

    try:
        from antenv.axon_hooks import get_axon_ntff_profile_hook
        return True
    except ImportError:
        pass
    try:
        import antenv
        from trn_agent_boot.trn_boot import _ntff_profile_via_ctypes

        hook = _ntff_profile_via_ctypes("/opt/axon/libaxon_pjrt.so")
        if hook is None:
            return False
        mod = types.ModuleType("antenv.axon_hooks")
        state = {"hook": hook}
        mod.get_axon_ntff_profile_hook = lambda: state["hook"]
        mod.set_axon_ntff_profile_hook = lambda h: state.update(hook=h)
        sys.modules["antenv.axon_hooks"] = mod
        antenv.axon_hooks = mod
        return True
    except Exception:
        return False


def _run(x, Wq, bq, Wk, bk, Wv, bv, mask, trace=False):
    if trace:
        trace = _install_ntff_hook()
    x = np.ascontiguousarray(np.asarray(x, np.float32))
    Wq = np.asarray(Wq, np.float32)
    Wk = np.asarray(Wk, np.float32)
    Wv = np.asarray(Wv, np.float32)
    bq = np.asarray(bq, np.float32)
    bk = np.asarray(bk, np.float32)
    bv = np.asarray(bv, np.float32)
    has_bias = bool(np.any(bq) or np.any(bk) or np.any(bv))
    has_mask = bool((np.asarray(mask) == 0).any())
    nc = _get_nc(has_bias, has_mask)
    maps = _in_maps(x, Wq, bq, Wk, bk, Wv, bv, mask, has_bias, has_mask)
    res = run_bass_kernel_spmd(nc, maps, list(range(N_CORES)), trace=trace)
    out = np.empty((B, S, D), np.float32)
    for c in range(N_CORES):
        b, g = divmod(c, N_CORES // B)
        out[b, :, g * DH:(g + 1) * DH] = res.results[c]["out"].T
    return out, res


def kernel(x, Wq, bq, Wk, bk, Wv, bv, mask):
    out, _ = _run(x, Wq, bq, Wk, bk, Wv, bv, mask)
    return out


# revision 20
# speedup vs baseline: 1.2443x; 1.0898x over previous
import sys

import ml_dtypes
import numpy as np

_TRN_REPO = "/opt/trn_rl_repo"
if _TRN_REPO not in sys.path:
    sys.path.insert(0, _TRN_REPO)

import concourse.tile as tile
from concourse import bacc, mybir
from concourse.bass_utils import run_bass_kernel_spmd

F32 = mybir.dt.float32
F32R = mybir.dt.float32r
BF16 = mybir.dt.bfloat16
AF = mybir.ActivationFunctionType

B, S, D = 2, 2048, 768
H_TOT, W = 12, 64
N_CORES = 8
HL = 3
DH = HL * W
KC = D // 128
ST = 512
NS = S // ST
PT = 1024
NT = S // 128
BF = ml_dtypes.bfloat16


def _round_f32r(a):
    u = np.ascontiguousarray(a, np.float32).view(np.uint32).copy()
    u += np.uint32(0x7FF) + ((u >> np.uint32(12)) & np.uint32(1))
    u &= np.uint32(0xFFFFF000)
    return u.view(np.float32)


def _emit(tc, aps, has_bias, has_mask):
    nc = tc.nc
    xt_d, wq_d, wk_d, wv_d, on_d, o32_d, mb_d, out_d = aps

    CH = 1 if has_mask else 3
    SCW = CH * ST

    from contextlib import ExitStack

    with ExitStack() as ctx:
        const = ctx.enter_context(tc.tile_pool(name="const", bufs=1))

        ones = None
        if has_bias:
            ones = const.tile([1, PT], BF16, name="ones", tag="ones")
        ones_r = const.tile([1, W], F32R, name="ones_r", tag="ones_r")
        mb = None
        if has_mask:
            mb = const.tile([128, NT], F32, name="mb", tag="mb")

        xt = []
        for c in range(KC):
            t = const.tile([128, S], BF16, name=f"xt{c}", tag=f"xt{c}")
            xt.append(t)

        def w_tiles(name):
            chunks = []
            for c in range(KC):
                t = const.tile([128, DH], BF16, name=f"{name}{c}",
                               tag=f"{name}{c}")
                chunks.append(t)
            brow = const.tile([1, DH], BF16, name=f"{name}b", tag=f"{name}b")
            return chunks, brow

        wq, wqb = w_tiles("wq")
        wk, wkb = w_tiles("wk")
        wv, wvb = w_tiles("wv")

        dmae = [nc.sync, nc.gpsimd]
        for c in range(KC):
            nc.scalar.dma_start(
                out=xt[c][:, 0:ST], in_=xt_d[c * 128:(c + 1) * 128, 0:ST])
            dmae[c % 2].dma_start(
                out=wk[c][:], in_=wk_d[c * 128:(c + 1) * 128, :])
        for c in range(KC):
            dmae[c % 2].dma_start(
                out=wq[c][:], in_=wq_d[c * 128:(c + 1) * 128, :])
        for si in range(1, NS):
            ssl = slice(si * ST, (si + 1) * ST)
            for c in range(KC):
                dmae[(si + c) % 2].dma_start(
                    out=xt[c][:, ssl], in_=xt_d[c * 128:(c + 1) * 128, ssl])
        for c in range(KC):
            dmae[c % 2].dma_start(
                out=wv[c][:], in_=wv_d[c * 128:(c + 1) * 128, :])
        if has_bias:
            for brow, w_d in ((wqb, wq_d), (wkb, wk_d), (wvb, wv_d)):
                nc.sync.dma_start(out=brow[:], in_=w_d[D:D + 1, :])
            nc.sync.dma_start(out=ones[:], in_=on_d[0:1, 0:PT])
        nc.sync.dma_start(out=ones_r[:], in_=o32_d[0:1, :])
        if has_mask:
            nc.sync.dma_start(out=mb[:], in_=mb_d[:, :])

        kt01 = const.tile([128, S], BF16, name="kt01", tag="kt01")
        qt01 = const.tile([128, S], BF16, name="qt01", tag="qt01")
        kt2 = const.tile([64, S], BF16, name="kt2", tag="kt2")
        qt2 = const.tile([64, S], BF16, name="qt2", tag="qt2")
        vaug = []
        for t in range(NT):
            va = const.tile([128, HL, W + 1], BF16, name=f"vaug{t}",
                            tag=f"vaug{t}")
            nc.gpsimd.memset(va[:, :, W:W + 1], 1.0)
            vaug.append(va)

        strm = ctx.enter_context(
            tc.tile_pool(name="strm", bufs=1, space="PSUM"))
        epi = ctx.enter_context(tc.tile_pool(name="epi", bufs=2))
        exp_pool = ctx.enter_context(tc.tile_pool(name="exp", bufs=6))

        def proj_main(si, dst, brow, wch):
            ssl = slice(si * ST, (si + 1) * ST)
            wrk = strm.tile([128, ST], F32, name="wrk", tag="work", bufs=2)
            for c in range(KC):
                nc.tensor.matmul(
                    wrk[:], wch[c][:, 0:128], xt[c][:, ssl],
                    start=(c == 0), stop=False, skip_group_check=True,
                )
            nc.tensor.matmul(
                wrk[:], brow[:, 0:128], ones[:, 0:ST],
                start=False, stop=True, skip_group_check=True,
            )
            nc.vector.tensor_copy(dst[:, ssl], wrk[:])

        def proj_h2_bias(si, wch, brow, dst):
            ssl = slice(si * ST, (si + 1) * ST)
            wrk = strm.tile([128, ST], F32, name="wrkb", tag="work", bufs=2)
            for c in range(KC):
                nc.tensor.matmul(
                    wrk[0:64, :], wch[c][:, 128:DH], xt[c][:, ssl],
                    start=(c == 0), stop=False, skip_group_check=True,
                )
            nc.tensor.matmul(
                wrk[0:64, :], brow[:, 128:DH], ones[:, 0:ST],
                start=False, stop=True, skip_group_check=True,
            )
            nc.vector.tensor_copy(dst[0:64, ssl], wrk[0:64, :])

        if has_bias:
            for si in range(NS):
                proj_main(si, kt01, wkb, wk)
                proj_h2_bias(si, wk, wkb, kt2)
                proj_main(si, qt01, wqb, wq)
                proj_h2_bias(si, wq, wqb, qt2)
            for t in range(NT):
                tsl = slice(t * 128, (t + 1) * 128)
                wrk = strm.tile([128, ST], F32, name="wrkv", tag="work",
                                bufs=2)
                for c in range(KC):
                    nc.tensor.matmul(
                        wrk[:, 0:DH], xt[c][:, tsl], wv[c][:],
                        start=(c == 0), stop=False, skip_group_check=True,
                    )
                nc.tensor.matmul(
                    wrk[:, 0:DH], ones[:, 0:128], wvb[:],
                    start=False, stop=True, skip_group_check=True,
                )
                nc.vector.tensor_copy(
                    vaug[t][:, :, 0:W],
                    wrk[:, 0:DH].rearrange("p (h w) -> p h w", h=HL),
                )

        st_ = {"sc": None, "used": 0, "base": 0, "chunks": [], "tick": 0,
               "keep": 2, "fill": 0, "pe": 0.0, "sch": 0}
        TARGET = 545.0

        def account(ns):
            st_["pe"] += ns
        stash = []
        pend1 = []
        pend2 = []

        def new_sc():
            st_["sc"] = strm.tile([128, SCW], F32, name="sc", tag="sc",
                                  bufs=2)
            st_["used"] = 0
            st_["base"] = 0
            st_["chunks"] = []

        def close_group():
            sc = st_["sc"]
            if sc is None:
                return
            n = len(st_["chunks"])
            if n:
                lo = st_["base"] * ST
                hi = lo + n * ST
                ex = exp_pool.tile([128, SCW], BF16, name="ex", tag="ex")
                tlast = st_["chunks"][-1][2]
                nc.scalar.activation(
                    ex[:, 0:n * ST], sc[:, lo:hi], AF.Exp,
                    bias=(mb[:, tlast:tlast + 1] if has_mask else 0.0),
                    scale=0.125,
                )
                stash.append(
                    (ex, [(cinfo, h, t, i * ST)
                          for i, (cinfo, h, t) in enumerate(st_["chunks"])]))
            st_["sc"] = None

        def emit_stash(drain=False):
            n_pop = len(stash) if drain else (
                1 if len(stash) > st_["keep"] else 0)
            for _ in range(n_pop):
                ex0, chunks0 = stash.pop(0)
                for (cinfo, h2, t2, col2) in chunks0:
                    nc.tensor.matmul(
                        cinfo["tile"][0:W + 1, :],
                        vaug[t2][:, h2, :],
                        ex0[:, col2:col2 + ST],
                        start=(t2 == 0), stop=(t2 == NT - 1),
                        skip_group_check=True,
                    )
                    cinfo["n"] += 1
                    account(235)

        def place_private(nslots):
            close_group()
            out = []
            for _ in range(nslots):
                if st_["sc"] is None or st_["used"] >= CH:
                    close_group()
                    new_sc()
                out.append((st_["sc"], st_["used"] * ST))
                st_["used"] += 1
                st_["base"] = st_["used"]
            if st_["used"] >= CH:
                st_["sc"] = None
            return out

        def place_score():
            if st_["sc"] is None or st_["used"] >= CH:
                close_group()
                new_sc()
            sc, col = st_["sc"], st_["used"] * ST
            st_["used"] += 1
            st_["sch"] += 1
            return sc, col

        def note_score(cinfo, h, t):
            st_["chunks"].append((cinfo, h, t))
            if st_["used"] >= CH:
                close_group()

        def pump_epi():
            st_["tick"] += 1
            if pend1 and pend1[0][0]["n"] == NT:
                _, p1, p2 = pend1.pop(0)
                p1()
                pend2.append((st_["tick"] + 4, p2))
            if pend2 and pend2[0][0] <= st_["tick"]:
                pend2.pop(0)[1]()

        def pump(ctx_t=None):
            emit_stash()
            pump_epi()
            if ctx_t is None:
                return
            deficit = st_["sch"] * TARGET - st_["pe"]
            if deficit > 60.0:
                n = min(512, max(128, int(deficit / 0.43)))
                nc.tensor.matmul(
                    ctx_t[96:128, 0:n],
                    kt01[0:64, 0:32], qt01[0:64, 0:n],
                    start=True, stop=True, skip_group_check=True,
                    tile_position=(0, 96),
                )
                account(n * 0.43 + 8)

        def epilogue(h, si, ctx_t):
            box = {}

            def p1():
                box["sumrow"] = epi.tile([1, ST], F32R, name="sumrow",
                                         tag="sumrow")
                box["ctx_sb"] = epi.tile([W, ST], F32R, name="ctx_sb",
                                         tag="ctx_sb")
                nc.vector.tensor_copy(box["sumrow"][:], ctx_t[W:W + 1, :])
                nc.vector.tensor_copy(box["ctx_sb"][:], ctx_t[0:W, :])

            def p2():
                nc.tensor.matmul(
                    ctx_t[0:W, :], ones_r[:], box["sumrow"][:],
                    start=True, stop=True, skip_group_check=True,
                )
                account(235)
                rc = epi.tile([W, ST], F32, name="rc", tag="rc")
                nc.vector.reciprocal_approx_fast(rc[:], ctx_t[0:W, :])
                ot = epi.tile([W, ST], F32, name="ot", tag="ot")
                nc.vector.tensor_mul(ot[:], box["ctx_sb"][:], rc[:])
                nc.sync.dma_start(
                    out=out_d[h * W:(h + 1) * W, si * ST:(si + 1) * ST],
                    in_=ot[:],
                )
            return p1, p2

        def priv_proj01(si, wch, dst):
            ssl = slice(si * ST, (si + 1) * ST)
            [(sc, col)] = place_private(1)
            for c in range(KC):
                nc.tensor.matmul(
                    sc[:, col:col + ST], wch[c][:, 0:128], xt[c][:, ssl],
                    start=(c == 0), stop=(c == KC - 1), skip_group_check=True,
                )
            account(1370)
            nc.vector.tensor_copy(dst[:, ssl], sc[:, col:col + ST])

        def priv_k(si):
            priv_proj01(si, wk, kt01)

        def priv_q01(si):
            priv_proj01(si, wq, qt01)

        def priv_b4(si):
            ssl = slice(si * ST, (si + 1) * ST)
            (scA, colA), (scB, colB) = place_private(2)
            pa = scA[:, colA:colA + ST]
            pb = scB[:, colB:colB + ST]
            for c in range(KC):
                stt, spp = (c == 0), (c == KC - 1)
                nc.tensor.matmul(
                    pa[0:64, :], wq[c][0:64, 128:DH], xt[c][0:64, ssl],
                    start=stt, stop=spp, skip_group_check=True,
                )
                nc.tensor.matmul(
                    pb[0:64, :], wq[c][64:128, 128:DH], xt[c][64:128, ssl],
                    start=stt, stop=spp, skip_group_check=True,
                )
                nc.tensor.matmul(
                    pa[64:128, :], wk[c][0:64, 128:DH], xt[c][0:64, ssl],
                    start=stt, stop=spp, skip_group_check=True,
                )
                nc.tensor.matmul(
                    pb[64:128, :], wk[c][64:128, 128:DH], xt[c][64:128, ssl],
                    start=stt, stop=spp, skip_group_check=True,
                )
            th = epi.tile([64, ST], F32, name="b4q", tag="b4q")
            nc.vector.tensor_copy(th[:], pa[0:64, :])
            nc.vector.tensor_add(qt2[:, ssl], th[:], pb[0:64, :])
            tk = epi.tile([64, ST], F32, name="b4k", tag="b4k")
            nc.vector.tensor_copy(tk[:], pa[64:128, :])
            nc.vector.tensor_add(kt2[:, ssl], tk[:], pb[64:128, :])
            account(1700)

        def priv_v(t):
            tsl = slice(t * 128, (t + 1) * 128)
            [(sc, col)] = place_private(1)
            pv = sc[:, col:col + DH]
            for c in range(KC):
                nc.tensor.matmul(
                    pv, xt[c][:, tsl], wv[c][:],
                    start=(c == 0), stop=(c == KC - 1), skip_group_check=True,
                )
            nc.vector.tensor_copy(
                vaug[t][:, :, 0:W],
                pv.rearrange("p (h w) -> p h w", h=HL),
            )
            account(543)

        def phase(h, si, privs, keep=2, fill=0):
            st_["keep"] = keep
            st_["fill"] = fill
            qsl = slice(si * ST, (si + 1) * ST)
            if h == 0:
                ktile, qtile, rows = kt01, qt01, slice(0, 64)
            elif h == 1:
                ktile, qtile, rows = kt01, qt01, slice(64, 128)
            else:
                ktile, qtile, rows = kt2, qt2, slice(0, 64)
            cinfo = {"tile": strm.tile([128, ST], F32, name="ctx",
                                       tag="work", bufs=2), "n": 0}
            for t in range(NT):
                for p in privs.get(t, ()):
                    p()
                sc, col = place_score()
                nc.tensor.matmul(
                    sc[:, col:col + ST],
                    ktile[rows, t * 128:(t + 1) * 128],
                    qtile[rows, qsl],
                    start=True, stop=True, skip_group_check=True,
                )
                account(228)
                note_score(cinfo, h, t)
                pump(cinfo["tile"])
            p1, p2 = epilogue(h, si, cinfo["tile"])
            pend1.append((cinfo, p1, p2))

        if not has_bias:
            priv_k(0)
            priv_q01(0)
            phase(0, 0, {2: [lambda: priv_v(0), lambda: priv_v(1)],
                         4: [lambda: priv_v(2), lambda: priv_v(3),
                             lambda: priv_k(1)],
                         6: [lambda: priv_v(4), lambda: priv_v(5)],
                         8: [lambda: priv_v(6), lambda: priv_v(7),
                             lambda: priv_k(2)],
                         10: [lambda: priv_k(3)],
                         13: [lambda: priv_q01(1)]},
                  keep=8)
            phase(0, 1, {0: [lambda: priv_v(8), lambda: priv_v(9)],
                         1: [lambda: priv_v(10), lambda: priv_v(11)],
                         2: [lambda: priv_v(12), lambda: priv_v(13)],
                         3: [lambda: priv_v(14), lambda: priv_v(15)],
                         12: [lambda: priv_q01(2)]})
            phase(0, 2, {12: [lambda: priv_q01(3)]})
            phase(0, 3, {})
            phase(1, 0, {4: [lambda: priv_b4(0)]})
            phase(1, 1, {4: [lambda: priv_b4(1)]})
            phase(1, 2, {4: [lambda: priv_b4(2)]})
            phase(1, 3, {4: [lambda: priv_b4(3)]})
            for si in range(NS):
                phase(2, si, {})
        else:
            for h in range(HL):
                for si in range(NS):
                    phase(h, si, {})
        close_group()
        emit_stash(drain=True)
        while pend1:
            _, p1, p2 = pend1.pop(0)
            p1()
            pend2.append((0, p2))
        while pend2:
            pend2.pop(0)[1]()


def _build(has_bias, has_mask):
    nc = bacc.Bacc(
        "TRN2", target_bir_lowering=False, debug=False, num_devices=N_CORES
    )
    xt_d = nc.dram_tensor("xt", [D, S], BF16, kind="ExternalInput").ap()
    wq_d = nc.dram_tensor("wq", [D + 1, DH], BF16, kind="ExternalInput").ap()
    wk_d = nc.dram_tensor("wk", [D + 1, DH], BF16, kind="ExternalInput").ap()
    wv_d = nc.dram_tensor("wv", [D + 1, DH], BF16, kind="ExternalInput").ap()
    on_d = nc.dram_tensor("onesd", [128, PT], BF16, kind="ExternalInput").ap()
    o32_d = nc.dram_tensor("ones32", [1, W], F32R, kind="ExternalInput").ap()
    mb_d = (
        nc.dram_tensor("mb", [128, NT], F32, kind="ExternalInput").ap()
        if has_mask else None
    )
    out_d = nc.dram_tensor("out", [DH, S], F32, kind="ExternalOutput").ap()

    with tile.TileContext(nc) as tc:
        _emit(tc, (xt_d, wq_d, wk_d, wv_d, on_d, o32_d, mb_d, out_d),
              has_bias, has_mask)
    nc.compile()
    return nc


_NC_CACHE = {}


def _get_nc(has_bias, has_mask):
    key = (has_bias, has_mask)
    if key not in _NC_CACHE:
        _NC_CACHE[key] = _build(has_bias, has_mask)
    return _NC_CACHE[key]


def _in_maps(x, Wq, bq, Wk, bk, Wv, bv, mask, has_bias, has_mask):
    xt_by_b = [np.ascontiguousarray(x[b].T).astype(BF) for b in range(B)]
    mb_by_b = [
        np.ascontiguousarray(
            ((np.asarray(mask[b]) == 0).astype(np.float32) * np.float32(-1e30))
            .reshape(NT, 128).T
        )
        for b in range(B)
    ]
    maps = []
    for c in range(N_CORES):
        b, g = divmod(c, N_CORES // B)
        lo = g * DH
        wq_a = np.empty((D + 1, DH), np.float32)
        wq_a[:D] = Wq[lo:lo + DH, :].T
        wq_a[D] = bq[lo:lo + DH]
        wk_a = np.empty((D + 1, DH), np.float32)
        wk_a[:D] = Wk[lo:lo + DH, :].T
        wk_a[D] = bk[lo:lo + DH]
        wv_a = np.empty((D + 1, DH), np.float32)
        wv_a[:D] = Wv[lo:lo + DH, :].T
        wv_a[D] = bv[lo:lo + DH]
        m = {
            "xt": xt_by_b[b], "wq": wq_a.astype(BF), "wk": wk_a.astype(BF),
            "wv": wv_a.astype(BF),
            "onesd": np.ones((128, PT), BF),
            "ones32": _round_f32r(np.ones((1, W), np.float32)),
        }
        if has_mask:
            m["mb"] = mb_by_b[b]
        maps.append(m)
    return maps


def _install_ntff_hook():
    import types

    try:
        from antenv.axon_hooks import get_axon_ntff_profile_hook
        return True
    except ImportError:
        pass
    try:
        import antenv
        from trn_agent_boot.trn_boot import _ntff_profile_via_ctypes

        hook = _ntff_profile_via_ctypes("/opt/axon/libaxon_pjrt.so")
        if hook is None:
            return False
        mod = types.ModuleType("antenv.axon_hooks")
        state = {"hook": hook}
        mod.get_axon_ntff_profile_hook = lambda: state["hook"]
        mod.set_axon_ntff_profile_hook = lambda h: state.update(hook=h)
        sys.modules["antenv.axon_hooks"] = mod
        antenv.axon_hooks = mod
        return True
    except Exception:
        return False


def _run(x, Wq, bq, Wk, bk, Wv, bv, mask, trace=False):
    if trace:
        trace = _install_ntff_hook()
    x = np.ascontiguousarray(np.asarray(x, np.float32))
    Wq = np.asarray(Wq, np.float32)
    Wk = np.asarray(Wk, np.float32)
    Wv = np.asarray(Wv, np.float32)
    bq = np.asarray(bq, np.float32)
    bk = np.asarray(bk, np.float32)
    bv = np.asarray(bv, np.float32)
    has_bias = bool(np.any(bq) or np.any(bk) or np.any(bv))
    has_mask = bool((np.asarray(mask) == 0).any())
    nc = _get_nc(has_bias, has_mask)
    maps = _in_maps(x, Wq, bq, Wk, bk, Wv, bv, mask, has_bias, has_mask)
    res = run_bass_kernel_spmd(nc, maps, list(range(N_CORES)), trace=trace)
    out = np.empty((B, S, D), np.float32)
    for c in range(N_CORES):
        b, g = divmod(c, N_CORES // B)
        out[b, :, g * DH:(g + 1) * DH] = res.results[c]["out"].T
    return out, res


def kernel(x, Wq, bq, Wk, bk, Wv, bv, mask):
    out, _ = _run(x, Wq, bq, Wk, bk, Wv, bv, mask)
    return out


# revision 21
# speedup vs baseline: 1.4065x; 1.1303x over previous
import sys

import ml_dtypes
import numpy as np

_TRN_REPO = "/opt/trn_rl_repo"
if _TRN_REPO not in sys.path:
    sys.path.insert(0, _TRN_REPO)

import concourse.tile as tile
from concourse import bacc, mybir
from concourse.bass_utils import run_bass_kernel_spmd

F32 = mybir.dt.float32
F32R = mybir.dt.float32r
BF16 = mybir.dt.bfloat16
AF = mybir.ActivationFunctionType

B, S, D = 2, 2048, 768
H_TOT, W = 12, 64
N_CORES = 8
HL = 3
DH = HL * W
KC = D // 128
ST = 512
NS = S // ST
PT = 1024
NT = S // 128
BF = ml_dtypes.bfloat16


def _round_f32r(a):
    u = np.ascontiguousarray(a, np.float32).view(np.uint32).copy()
    u += np.uint32(0x7FF) + ((u >> np.uint32(12)) & np.uint32(1))
    u &= np.uint32(0xFFFFF000)
    return u.view(np.float32)


def _emit(tc, aps, has_bias, has_mask):
    nc = tc.nc
    xt_d, wq_d, wk_d, wv_d, on_d, o32_d, mb_d, out_d = aps

    CH = 1 if has_mask else 3
    SCW = CH * ST

    from contextlib import ExitStack

    with ExitStack() as ctx:
        const = ctx.enter_context(tc.tile_pool(name="const", bufs=1))

        ones = None
        if has_bias:
            ones = const.tile([1, PT], BF16, name="ones", tag="ones")
        ones_r = const.tile([1, W], F32R, name="ones_r", tag="ones_r")
        mb = None
        if has_mask:
            mb = const.tile([128, NT], F32, name="mb", tag="mb")

        xt = []
        for c in range(KC):
            t = const.tile([128, S], BF16, name=f"xt{c}", tag=f"xt{c}")
            xt.append(t)

        def w_tiles(name):
            chunks = []
            for c in range(KC):
                t = const.tile([128, DH], BF16, name=f"{name}{c}",
                               tag=f"{name}{c}")
                chunks.append(t)
            brow = const.tile([1, DH], BF16, name=f"{name}b", tag=f"{name}b")
            return chunks, brow

        wq, wqb = w_tiles("wq")
        wk, wkb = w_tiles("wk")
        wv, wvb = w_tiles("wv")

        dmae = [nc.sync, nc.gpsimd]
        for c in range(KC):
            nc.scalar.dma_start(
                out=xt[c][:, 0:ST], in_=xt_d[c * 128:(c + 1) * 128, 0:ST])
            dmae[c % 2].dma_start(
                out=wk[c][:], in_=wk_d[c * 128:(c + 1) * 128, :])
        for c in range(KC):
            dmae[c % 2].dma_start(
                out=wq[c][:], in_=wq_d[c * 128:(c + 1) * 128, :])
        for si in range(1, NS):
            ssl = slice(si * ST, (si + 1) * ST)
            for c in range(KC):
                dmae[(si + c) % 2].dma_start(
                    out=xt[c][:, ssl], in_=xt_d[c * 128:(c + 1) * 128, ssl])
        for c in range(KC):
            dmae[c % 2].dma_start(
                out=wv[c][:], in_=wv_d[c * 128:(c + 1) * 128, :])
        if has_bias:
            for brow, w_d in ((wqb, wq_d), (wkb, wk_d), (wvb, wv_d)):
                nc.sync.dma_start(out=brow[:], in_=w_d[D:D + 1, :])
            nc.sync.dma_start(out=ones[:], in_=on_d[0:1, 0:PT])
        nc.sync.dma_start(out=ones_r[:], in_=o32_d[0:1, :])
        if has_mask:
            nc.sync.dma_start(out=mb[:], in_=mb_d[:, :])

        kt01 = const.tile([128, S], BF16, name="kt01", tag="kt01")
        qt01 = const.tile([128, S], BF16, name="qt01", tag="qt01")
        kt2 = const.tile([64, S], BF16, name="kt2", tag="kt2")
        qt2 = const.tile([64, S], BF16, name="qt2", tag="qt2")
        vaug = []
        for t in range(NT):
            va = const.tile([128, HL, W + 1], BF16, name=f"vaug{t}",
                            tag=f"vaug{t}")
            nc.gpsimd.memset(va[:, :, W:W + 1], 1.0)
            vaug.append(va)

        strm = ctx.enter_context(
            tc.tile_pool(name="strm", bufs=1, space="PSUM"))
        epi = ctx.enter_context(tc.tile_pool(name="epi", bufs=2))
        exp_pool = ctx.enter_context(tc.tile_pool(name="exp", bufs=12))

        def proj_main(si, dst, brow, wch):
            ssl = slice(si * ST, (si + 1) * ST)
            wrk = strm.tile([128, ST], F32, name="wrk", tag="work", bufs=2)
            for c in range(KC):
                nc.tensor.matmul(
                    wrk[:], wch[c][:, 0:128], xt[c][:, ssl],
                    start=(c == 0), stop=False, skip_group_check=True,
                )
            nc.tensor.matmul(
                wrk[:], brow[:, 0:128], ones[:, 0:ST],
                start=False, stop=True, skip_group_check=True,
            )
            nc.vector.tensor_copy(dst[:, ssl], wrk[:])

        def proj_h2_bias(si, wch, brow, dst):
            ssl = slice(si * ST, (si + 1) * ST)
            wrk = strm.tile([128, ST], F32, name="wrkb", tag="work", bufs=2)
            for c in range(KC):
                nc.tensor.matmul(
                    wrk[0:64, :], wch[c][:, 128:DH], xt[c][:, ssl],
                    start=(c == 0), stop=False, skip_group_check=True,
                )
            nc.tensor.matmul(
                wrk[0:64, :], brow[:, 128:DH], ones[:, 0:ST],
                start=False, stop=True, skip_group_check=True,
            )
            nc.vector.tensor_copy(dst[0:64, ssl], wrk[0:64, :])

        if has_bias:
            for si in range(NS):
                proj_main(si, kt01, wkb, wk)
                proj_h2_bias(si, wk, wkb, kt2)
                proj_main(si, qt01, wqb, wq)
                proj_h2_bias(si, wq, wqb, qt2)
            for t in range(NT):
                tsl = slice(t * 128, (t + 1) * 128)
                wrk = strm.tile([128, ST], F32, name="wrkv", tag="work",
                                bufs=2)
                for c in range(KC):
                    nc.tensor.matmul(
                        wrk[:, 0:DH], xt[c][:, tsl], wv[c][:],
                        start=(c == 0), stop=False, skip_group_check=True,
                    )
                nc.tensor.matmul(
                    wrk[:, 0:DH], ones[:, 0:128], wvb[:],
                    start=False, stop=True, skip_group_check=True,
                )
                nc.vector.tensor_copy(
                    vaug[t][:, :, 0:W],
                    wrk[:, 0:DH].rearrange("p (h w) -> p h w", h=HL),
                )

        st_ = {"sc": None, "used": 0, "base": 0, "chunks": [], "tick": 0,
               "keep": 2, "fill": 0, "pe": 0.0, "sch": 0}
        TARGET = 590.0

        def account(ns):
            st_["pe"] += ns
        stash = []
        pend1 = []
        pend2 = []

        def new_sc():
            st_["sc"] = strm.tile([128, SCW], F32, name="sc", tag="sc",
                                  bufs=2)
            st_["used"] = 0
            st_["base"] = 0
            st_["chunks"] = []

        def close_group():
            sc = st_["sc"]
            if sc is None:
                return
            n = len(st_["chunks"])
            if n:
                lo = st_["base"] * ST
                hi = lo + n * ST
                ex = exp_pool.tile([128, SCW], BF16, name="ex", tag="ex")
                tlast = st_["chunks"][-1][2]
                nc.scalar.activation(
                    ex[:, 0:n * ST], sc[:, lo:hi], AF.Exp,
                    bias=(mb[:, tlast:tlast + 1] if has_mask else 0.0),
                    scale=0.125,
                )
                stash.append(
                    (ex, [(cinfo, h, t, i * ST)
                          for i, (cinfo, h, t) in enumerate(st_["chunks"])]))
            st_["sc"] = None

        def emit_stash(drain=False):
            n_pop = len(stash) if drain else (
                1 if len(stash) > st_["keep"] else 0)
            for _ in range(n_pop):
                ex0, chunks0 = stash.pop(0)
                for (cinfo, h2, t2, col2) in chunks0:
                    nc.tensor.matmul(
                        cinfo["tile"][0:W + 1, :],
                        vaug[t2][:, h2, :],
                        ex0[:, col2:col2 + ST],
                        start=(t2 == 0), stop=(t2 == NT - 1),
                        skip_group_check=True,
                    )
                    cinfo["n"] += 1
                    account(235)

        def place_private(nslots):
            close_group()
            out = []
            for _ in range(nslots):
                if st_["sc"] is None or st_["used"] >= CH:
                    close_group()
                    new_sc()
                out.append((st_["sc"], st_["used"] * ST))
                st_["used"] += 1
                st_["base"] = st_["used"]
            if st_["used"] >= CH:
                st_["sc"] = None
            return out

        def place_score():
            if st_["sc"] is None or st_["used"] >= CH:
                close_group()
                new_sc()
            sc, col = st_["sc"], st_["used"] * ST
            st_["used"] += 1
            st_["sch"] += 1
            return sc, col

        def note_score(cinfo, h, t):
            st_["chunks"].append((cinfo, h, t))
            if st_["used"] >= CH:
                close_group()

        def pump_epi():
            st_["tick"] += 1
            if pend1 and pend1[0][0]["n"] == NT:
                _, p1, p2 = pend1.pop(0)
                p1()
                pend2.append((st_["tick"] + 4, p2))
            if pend2 and pend2[0][0] <= st_["tick"]:
                pend2.pop(0)[1]()

        def pump(ctx_t=None):
            emit_stash()
            pump_epi()
            if ctx_t is None:
                return
            deficit = st_["sch"] * TARGET - st_["pe"]
            if deficit > 60.0:
                n = min(512, max(128, int(deficit / 0.43)))
                nc.tensor.matmul(
                    ctx_t[96:128, 0:n],
                    kt01[0:64, 0:32], qt01[0:64, 0:n],
                    start=True, stop=True, skip_group_check=True,
                    tile_position=(0, 96),
                )
                account(n * 0.43 + 8)

        def epilogue(h, si, ctx_t):
            box = {}

            def p1():
                box["sumrow"] = epi.tile([1, ST], F32R, name="sumrow",
                                         tag="sumrow")
                box["ctx_sb"] = epi.tile([W, ST], F32R, name="ctx_sb",
                                         tag="ctx_sb")
                nc.vector.tensor_copy(box["sumrow"][:], ctx_t[W:W + 1, :])
                nc.vector.tensor_copy(box["ctx_sb"][:], ctx_t[0:W, :])

            def p2():
                nc.tensor.matmul(
                    ctx_t[0:W, :], ones_r[:], box["sumrow"][:],
                    start=True, stop=True, skip_group_check=True,
                )
                account(235)
                rc = epi.tile([W, ST], F32, name="rc", tag="rc")
                nc.vector.reciprocal_approx_fast(rc[:], ctx_t[0:W, :])
                ot = epi.tile([W, ST], F32, name="ot", tag="ot")
                nc.vector.tensor_mul(ot[:], box["ctx_sb"][:], rc[:])
                nc.sync.dma_start(
                    out=out_d[h * W:(h + 1) * W, si * ST:(si + 1) * ST],
                    in_=ot[:],
                )
            return p1, p2

        def priv_proj01(si, wch, dst):
            ssl = slice(si * ST, (si + 1) * ST)
            [(sc, col)] = place_private(1)
            for c in range(KC):
                nc.tensor.matmul(
                    sc[:, col:col + ST], wch[c][:, 0:128], xt[c][:, ssl],
                    start=(c == 0), stop=(c == KC - 1), skip_group_check=True,
                )
            account(1370)
            nc.vector.tensor_copy(dst[:, ssl], sc[:, col:col + ST])

        def priv_k(si):
            priv_proj01(si, wk, kt01)

        def priv_q01(si):
            priv_proj01(si, wq, qt01)

        def priv_proj_h2(si, wch, dst):
            ssl = slice(si * ST, (si + 1) * ST)
            [(sc, col)] = place_private(1)
            for c in range(KC):
                nc.tensor.matmul(
                    sc[0:64, col:col + ST], wch[c][:, 128:DH],
                    xt[c][:, ssl],
                    start=(c == 0), stop=(c == KC - 1), skip_group_check=True,
                )
            account(1330)
            nc.vector.tensor_copy(dst[:, ssl], sc[0:64, col:col + ST])

        def priv_q2(si):
            priv_proj_h2(si, wq, qt2)

        def priv_k2(si):
            priv_proj_h2(si, wk, kt2)

        def priv_v(t):
            tsl = slice(t * 128, (t + 1) * 128)
            [(sc, col)] = place_private(1)
            pv = sc[:, col:col + DH]
            for c in range(KC):
                nc.tensor.matmul(
                    pv, xt[c][:, tsl], wv[c][:],
                    start=(c == 0), stop=(c == KC - 1), skip_group_check=True,
                )
            nc.vector.tensor_copy(
                vaug[t][:, :, 0:W],
                pv.rearrange("p (h w) -> p h w", h=HL),
            )
            account(543)

        def phase(h, si, privs, keep=2, fill=0):
            st_["keep"] = keep
            st_["fill"] = fill
            qsl = slice(si * ST, (si + 1) * ST)
            if h == 0:
                ktile, qtile, rows = kt01, qt01, slice(0, 64)
            elif h == 1:
                ktile, qtile, rows = kt01, qt01, slice(64, 128)
            else:
                ktile, qtile, rows = kt2, qt2, slice(0, 64)
            cinfo = {"tile": strm.tile([128, ST], F32, name="ctx",
                                       tag="work", bufs=2), "n": 0}
            for t in range(NT):
                for p in privs.get(t, ()):
                    p()
                sc, col = place_score()
                nc.tensor.matmul(
                    sc[:, col:col + ST],
                    ktile[rows, t * 128:(t + 1) * 128],
                    qtile[rows, qsl],
                    start=True, stop=True, skip_group_check=True,
                )
                account(228)
                note_score(cinfo, h, t)
                pump(cinfo["tile"])
            p1, p2 = epilogue(h, si, cinfo["tile"])
            pend1.append((cinfo, p1, p2))

        if not has_bias:
            priv_k(0)
            priv_q01(0)
            phase(0, 0, {2: [lambda: priv_v(0), lambda: priv_v(1)],
                         3: [lambda: priv_k(1)],
                         4: [lambda: priv_v(2), lambda: priv_v(3)],
                         6: [lambda: priv_q01(1)],
                         7: [lambda: priv_v(4), lambda: priv_v(5)],
                         8: [lambda: priv_k(2)],
                         9: [lambda: priv_v(6), lambda: priv_v(7)],
                         10: [lambda: priv_k(3)]},
                  keep=8)
            phase(0, 1, {0: [lambda: priv_v(8), lambda: priv_v(9)],
                         1: [lambda: priv_v(10), lambda: priv_v(11)],
                         2: [lambda: priv_v(12), lambda: priv_v(13)],
                         3: [lambda: priv_v(14), lambda: priv_v(15)],
                         8: [lambda: priv_q2(0), lambda: priv_k2(0)],
                         12: [lambda: priv_q01(2)]})
            phase(0, 2, {6: [lambda: priv_q2(1), lambda: priv_k2(1)],
                         12: [lambda: priv_q01(3)]})
            phase(0, 3, {6: [lambda: priv_q2(2), lambda: priv_k2(2)]})
            phase(1, 0, {6: [lambda: priv_q2(3), lambda: priv_k2(3)]})
            for si in range(1, NS):
                phase(1, si, {})
            for si in range(NS):
                phase(2, si, {})
        else:
            for h in range(HL):
                for si in range(NS):
                    phase(h, si, {})
        close_group()
        emit_stash(drain=True)
        while pend1:
            _, p1, p2 = pend1.pop(0)
            p1()
            pend2.append((0, p2))
        while pend2:
            pend2.pop(0)[1]()


def _build(has_bias, has_mask):
    nc = bacc.Bacc(
        "TRN2", target_bir_lowering=False, debug=False, num_devices=N_CORES
    )
    xt_d = nc.dram_tensor("xt", [D, S], BF16, kind="ExternalInput").ap()
    wq_d = nc.dram_tensor("wq", [D + 1, DH], BF16, kind="ExternalInput").ap()
    wk_d = nc.dram_tensor("wk", [D + 1, DH], BF16, kind="ExternalInput").ap()
    wv_d = nc.dram_tensor("wv", [D + 1, DH], BF16, kind="ExternalInput").ap()
    on_d = nc.dram_tensor("onesd", [128, PT], BF16, kind="ExternalInput").ap()
    o32_d = nc.dram_tensor("ones32", [1, W], F32R, kind="ExternalInput").ap()
    mb_d = (
        nc.dram_tensor("mb", [128, NT], F32, kind="ExternalInput").ap()
        if has_mask else None
    )
    out_d = nc.dram_tensor("out", [DH, S], F32, kind="ExternalOutput").ap()

    with tile.TileContext(nc) as tc:
        _emit(tc, (xt_d, wq_d, wk_d, wv_d, on_d, o32_d, mb_d, out_d),
              has_bias, has_mask)
    nc.compile()
    return nc


_NC_CACHE = {}


def _get_nc(has_bias, has_mask):
    key = (has_bias, has_mask)
    if key not in _NC_CACHE:
        _NC_CACHE[key] = _build(has_bias, has_mask)
    return _NC_CACHE[key]


def _in_maps(x, Wq, bq, Wk, bk, Wv, bv, mask, has_bias, has_mask):
    xt_by_b = [np.ascontiguousarray(x[b].T).astype(BF) for b in range(B)]
    mb_by_b = [
        np.ascontiguousarray(
            ((np.asarray(mask[b]) == 0).astype(np.float32) * np.float32(-1e30))
            .reshape(NT, 128).T
        )
        for b in range(B)
    ]
    maps = []
    for c in range(N_CORES):
        b, g = divmod(c, N_CORES // B)
        lo = g * DH
        wq_a = np.empty((D + 1, DH), np.float32)
        wq_a[:D] = Wq[lo:lo + DH, :].T
        wq_a[D] = bq[lo:lo + DH]
        wk_a = np.empty((D + 1, DH), np.float32)
        wk_a[:D] = Wk[lo:lo + DH, :].T
        wk_a[D] = bk[lo:lo + DH]
        wv_a = np.empty((D + 1, DH), np.float32)
        wv_a[:D] = Wv[lo:lo + DH, :].T
        wv_a[D] = bv[lo:lo + DH]
        m = {
            "xt": xt_by_b[b], "wq": wq_a.astype(BF), "wk": wk_a.astype(BF),
            "wv": wv_a.astype(BF),
            "onesd": np.ones((128, PT), BF),
            "ones32": _round_f32r(np.ones((1, W), np.float32)),
        }
        if has_mask:
            m["mb"] = mb_by_b[b]
        maps.append(m)
    return maps


def _install_ntff_hook():
    import types

    try:
        from antenv.axon_hooks import get_axon_ntff_profile_hook
        return True
    except ImportError:
        pass
    try:
        import antenv
        from trn_agent_boot.trn_boot import _ntff_profile_via_ctypes

        hook = _ntff_profile_via_ctypes("/opt/axon/libaxon_pjrt.so")
        if hook is None:
            return False
        mod = types.ModuleType("antenv.axon_hooks")
        state = {"hook": hook}
        mod.get_axon_ntff_profile_hook = lambda: state["hook"]
        mod.set_axon_ntff_profile_hook = lambda h: state.update(hook=h)
        sys.modules["antenv.axon_hooks"] = mod
        antenv.axon_hooks = mod
        return True
    except Exception:
        return False


def _run(x, Wq, bq, Wk, bk, Wv, bv, mask, trace=False):
    if trace:
        trace = _install_ntff_hook()
    x = np.ascontiguousarray(np.asarray(x, np.float32))
    Wq = np.asarray(Wq, np.float32)
    Wk = np.asarray(Wk, np.float32)
    Wv = np.asarray(Wv, np.float32)
    bq = np.asarray(bq, np.float32)
    bk = np.asarray(bk, np.float32)
    bv = np.asarray(bv, np.float32)
    has_bias = bool(np.any(bq) or np.any(bk) or np.any(bv))
    has_mask = bool((np.asarray(mask) == 0).any())
    nc = _get_nc(has_bias, has_mask)
    maps = _in_maps(x, Wq, bq, Wk, bk, Wv, bv, mask, has_bias, has_mask)
    res = run_bass_kernel_spmd(nc, maps, list(range(N_CORES)), trace=trace)
    out = np.empty((B, S, D), np.float32)
    for c in range(N_CORES):
        b, g = divmod(c, N_CORES // B)
        out[b, :, g * DH:(g + 1) * DH] = res.results[c]["out"].T
    return out, res


def kernel(x, Wq, bq, Wk, bk, Wv, bv, mask):
    out, _ = _run(x, Wq, bq, Wk, bk, Wv, bv, mask)
    return out
